# revision 29
# baseline (speedup 1.0000x reference)
"""Causal self-attention (GPT-2 small block shape: B=4, T=2048, C=768, H=12, D=64)
on 8 TRN2 NeuronCores.

Sharding: core i handles batch b = i//2 and head-half = i%2 (6 heads each).
No cross-core collectives; the two half-head partial output projections per
batch are summed on the host during unshard (row-parallel c_proj).

Device kernel (per core, all matmuls bf16, fp32 PSUM accumulation), v2:
  - The two heads of a pair run as CONCURRENT row-tiled S matmuls (K=64 at
    PE row offsets 0/64) writing the two 512-col halves of one [128,1024]
    PSUM tile, so S-gen runs at full PE rate.
  - One exp per (pair, q-block, k-chunk) covers both heads; causally
    trimmed via a 3-D access pattern.  ScalarE(exp) and TensorE are the
    two co-critical engines (~125 us each); everything else hides.
  - Flattened software-pipelined emission: AV lags S by a few steps and
    QKV / output-projection matmuls are split into ~1 us filler units
    woven between attention steps (the Tile list scheduler uses program
    order as priority).
  - Softmax normalization: ones-column in V accumulates row sums into PSUM
    partition 64; reciprocal straight off PSUM, one partition-broadcast per
    pair, two DVE multiplies.
  - Output written bf16 (summed fp32 on host).
"""

import sys

if "/opt/trn_rl_repo" not in sys.path:
    sys.path.insert(0, "/opt/trn_rl_repo")

import numpy as np
import ml_dtypes

import concourse.bass as bass  # noqa: F401
import concourse.mybir as mybir
from concourse import bacc
from concourse.tile import TileContext
from concourse.bass_utils import run_bass_kernel_spmd

BF16 = ml_dtypes.bfloat16

B, T, C = 4, 2048, 768
H, D = 12, 64
NH = 6  # heads per core
P = 128
TC = T // P  # 16 t-chunks of 128
QC = T // 512  # 4 q-blocks of 512
CCH = C // P  # 6 contraction chunks

DT = mybir.dt.bfloat16
F32 = mybir.dt.float32
F8 = mybir.dt.float8e4
F8NP = ml_dtypes.float8_e4m3
EXP_SCALE = 1.0 / 8192.0  # undo the x32 fp8 pre-scales and 1/sqrt(D)


def build_nc():
    nc = bacc.Bacc()

    # t-block-major so each 512-col block is one per-partition-contiguous DMA
    xt_d = nc.declare_dram_parameter("xt", [P, QC, CCH, 512], DT, isOutput=False)
    # pair-major: wqk[p, pair, cc, 0:128]=Q chunk, [..., 128:256]=K chunk —
    # per-partition-contiguous per pair so each pair is one full-rate DMA
    wqk_d = nc.declare_dram_parameter("wqk", [P, 3, CCH, 2 * P], DT, isOutput=False)
    bqk_d = nc.declare_dram_parameter("bqk", [P, 2 * NH * D // P], F32, isOutput=False)
    wv_d = nc.declare_dram_parameter("wv", [P, CCH, NH * D], DT, isOutput=False)
    bv_d = nc.declare_dram_parameter("bv", [1, NH * D], DT, isOutput=False)
    wp_d = nc.declare_dram_parameter("wp", [P, NH * D // P, C], DT, isOutput=False)
    bp_d = nc.declare_dram_parameter("bp", [1, C], DT, isOutput=False)
    mask_d = nc.declare_dram_parameter("mask", [P, P], DT, isOutput=False)
    out_d = nc.declare_dram_parameter("out", [T, C], DT, isOutput=True)

    with TileContext(nc) as tc:
        with (
            tc.tile_pool(name="consts", bufs=1) as consts,
            tc.tile_pool(name="sexp", bufs=6) as sexp_pool,
            tc.tile_pool(name="inv", bufs=2) as inv_pool,
            tc.tile_pool(name="invb", bufs=2) as invb_pool,
            tc.tile_pool(name="outp", bufs=4) as outp,
            tc.tile_pool(name="ps_s", bufs=2, space="PSUM") as ps_s,
            tc.tile_pool(name="ps_av", bufs=2, space="PSUM") as ps_av,
            tc.tile_pool(name="ps_mm", bufs=2, space="PSUM") as ps_mm,
        ):
            # ---- input DMAs, one queue (a single InstDMACopy spreads over
            # all 16 SDMA engines; a second ring does not add HBM bandwidth),
            # issued on sync so the ACT queue stays clean for exps.
            # Ordered by first use.
            wqk_sb = consts.tile([P, 3, CCH, 2 * P], DT)
            nc.sync.dma_start(wqk_sb[:, 0], wqk_d[:, 0])
            bqk_sb = consts.tile([P, 2 * NH * D // P], F32)
            nc.sync.dma_start(bqk_sb[:], bqk_d[:])
            xt_sb = consts.tile([P, QC, CCH, 512], DT)
            nc.sync.dma_start(xt_sb[:, 0], xt_d[:, 0])
            nc.sync.dma_start(wqk_sb[:, 1], wqk_d[:, 1])
            nc.sync.dma_start(wqk_sb[:, 2], wqk_d[:, 2])
            mask_sb = consts.tile([P, P], DT)
            nc.sync.dma_start(mask_sb[:], mask_d[:])
            wv_sb = consts.tile([P, CCH, NH * D], DT)
            nc.sync.dma_start(wv_sb[:], wv_d[:])
            bv_sb = consts.tile([1, NH * D], DT)
            nc.sync.dma_start(bv_sb[:], bv_d[:])
            nc.sync.dma_start(xt_sb[:, 1], xt_d[:, 1])
            nc.sync.dma_start(xt_sb[:, 2], xt_d[:, 2])
            nc.sync.dma_start(xt_sb[:, 3], xt_d[:, 3])
            wp_sb = consts.tile([P, NH * D // P, C], DT)
            nc.sync.dma_start(wp_sb[:], wp_d[:])
            bp_sb = consts.tile([1, C], DT)
            nc.sync.dma_start(bp_sb[:], bp_d[:])

            # pre-warm the ACT exp table during the DMA-bound ramp
            warm = consts.tile([1, 8], F32)
            nc.gpsimd.memset(warm[:], 0.0)
            nc.scalar.activation(warm[:], warm[:], mybir.ActivationFunctionType.Exp)

            bvb = consts.tile([P, NH * D], DT)
            nc.gpsimd.partition_broadcast(bvb[:], bv_sb[:])
            bpb = consts.tile([P, C], DT)
            nc.gpsimd.partition_broadcast(bpb[:], bp_sb[:])

            # Q^T/K^T as head-pair tiles [128, T]: head 2p in partitions 0:64,
            # head 2p+1 in partitions 64:128.
            qtp = [consts.tile([P, T], DT, name=f"qtp{p}", tag=f"qtp{p}") for p in range(3)]
            ktp = [consts.tile([P, T], DT, name=f"ktp{p}", tag=f"ktp{p}") for p in range(3)]
            # V per t-chunk, heads side by side with a ones column: [128, 6, 65]
            vt = [consts.tile([P, NH, D + 1], DT, name=f"vt{t}", tag=f"vt{t}") for t in range(TC)]
            for t in range(TC):
                nc.gpsimd.memset(vt[t][:, :, D : D + 1], 1.0)
            # y^T per head-pair [128, T] bf16 (unnormalized until norm step)
            yt = [consts.tile([P, T], DT, name=f"yt{p}", tag=f"yt{p}") for p in range(3)]

            # ---- filler units (QKV projection / output projection) ----
            def qk_unit(fc, r):
                # feature chunk fc: 0..2 -> Q pair fc, 3..5 -> K pair fc-3
                pair, koff = (fc, 0) if fc < 3 else (fc - 3, P)
                pq = ps_mm.tile([P, 512], F32, tag="mm", name="pq")
                for cc in range(CCH):
                    nc.tensor.matmul(
                        pq[:],
                        wqk_sb[:, pair, cc, koff : koff + P],
                        xt_sb[:, r, cc, :],
                        start=(cc == 0),
                        stop=(cc == CCH - 1),
                        skip_group_check=True,
                    )
                dst = qtp[fc] if fc < 3 else ktp[fc - 3]
                nc.vector.tensor_scalar_add(
                    dst[:, r * 512 : (r + 1) * 512],
                    pq[:],
                    bqk_sb[:, fc : fc + 1],
                )

            def v_unit(t):
                pv = ps_mm.tile([P, NH * D], F32, tag="mm", name="pv")
                for cc in range(CCH):
                    nc.tensor.matmul(
                        pv[:],
                        xt_sb[:, t // 4, cc, (t % 4) * P : (t % 4 + 1) * P],
                        wv_sb[:, cc, :],
                        start=(cc == 0),
                        stop=(cc == CCH - 1),
                        skip_group_check=True,
                    )
                nc.vector.tensor_add(
                    vt[t][:, :, 0:D],
                    pv[:].rearrange("p (h d) -> p h d", d=D),
                    bvb[:].rearrange("p (h d) -> p h d", d=D),
                )

            proj_stg = {}

            def proj_unit(t, half):
                # halves share one [128, C] staging tile; half 1 sends the
                # whole row-block as a single contiguous DMA
                c0, c1 = (0, 384) if half == 0 else (384, C)
                pp = ps_mm.tile([P, c1 - c0], F32, tag="mm", name="pp")
                for cp in range(3):
                    nc.tensor.matmul(
                        pp[:],
                        yt[cp][:, t * P : (t + 1) * P],
                        wp_sb[:, cp, c0:c1],
                        start=(cp == 0),
                        stop=(cp == 2),
                        skip_group_check=True,
                    )
                if half == 0:
                    proj_stg[t] = outp.tile([P, C], DT, name="stg", tag="stg")
                stg = proj_stg[t]
                nc.vector.tensor_add(stg[:, c0:c1], pp[:], bpb[:, c0:c1])
                if half == 1:
                    nc.sync.dma_start(out_d[t * P : (t + 1) * P, :], stg[:])

            # ---- attention step pieces ----
            # per-block state: av tiles (h0, h1) and the step list
            def s_pair(hp, qc, j, psS):
                m = max(0, (j - 4 * qc) * P)
                js = slice(j * P, (j + 1) * P)
                qs = slice(qc * 512 + m, (qc + 1) * 512)
                nc.tensor.matmul(
                    psS[:, m:512],
                    ktp[hp][0:64, js],
                    qtp[hp][0:64, qs],
                    start=True,
                    stop=True,
                )
                nc.tensor.matmul(
                    psS[:, 512 + m : 1024],
                    ktp[hp][64:128, js],
                    qtp[hp][64:128, qs],
                    start=True,
                    stop=True,
                )

            def exp_pair(hp, qc, j, psS, sexp):
                m = max(0, (j - 4 * qc) * P)
                if m:
                    src = psS[:].rearrange("p (s q) -> p s q", s=2)[:, :, m:512]
                    dst = sexp[:].rearrange("p (s q) -> p s q", s=2)[:, :, m:512]
                else:
                    src, dst = psS[:], sexp[:]
                nc.scalar.activation(dst, src, mybir.ActivationFunctionType.Exp)

            def mask_pair(hp, qc, j, sexp):
                m = (j - 4 * qc) * P
                for s in (0, 1):
                    nc.vector.tensor_mul(
                        sexp[:, s * 512 + m : s * 512 + m + P],
                        sexp[:, s * 512 + m : s * 512 + m + P],
                        mask_sb[:],
                    )

            def av_pair(hp, qc, j, nj, sexp, av0, av1):
                m = max(0, (j - 4 * qc) * P)
                for s, av in ((0, av0), (1, av1)):
                    nc.tensor.matmul(
                        av[:, m:512],
                        vt[j][:, 2 * hp + s, :],
                        sexp[:, s * 512 + m : (s + 1) * 512],
                        start=(j == 0),
                        stop=(j == nj - 1),
                        skip_group_check=True,
                    )

            def norm(hp, qc, av0, av1):
                # reciprocal_approx_fast misreads PSUM at partition offset 64;
                # stage the sums rows through SBUF first (copies are cheap).
                sums = inv_pool.tile([1, 1024], F32, name="sums", tag="sums")
                nc.vector.tensor_copy(sums[:, 0:512], av0[64:65, :])
                nc.vector.tensor_copy(sums[:, 512:1024], av1[64:65, :])
                inv = inv_pool.tile([1, 1024], F32, name="inv", tag="inv")
                nc.vector.reciprocal_approx_fast(inv[:], sums[:])
                invb = invb_pool.tile([64, 1024], F32, name="invb")
                nc.gpsimd.partition_broadcast(invb[:], inv[:])
                qs = slice(qc * 512, (qc + 1) * 512)
                nc.vector.tensor_mul(yt[hp][0:64, qs], av0[0:64, :], invb[:, 0:512])
                nc.vector.tensor_mul(yt[hp][64:128, qs], av1[0:64, :], invb[:, 512:1024])

            # ---- flattened pipeline ----
            LAG = 4
            steps = []
            for qc in range(QC):
                for hp in range(3):
                    nj = 4 * (qc + 1)
                    for j in range(nj):
                        steps.append((hp, qc, j, nj))
            # swap the last step of each block with the first of the next:
            # the new block's S-pair then reuses a psS slot freed two exps
            # earlier, removing the serial psS->S->exp bubble at boundaries
            block_starts = []
            acc = 0
            for qc in range(QC):
                for hp in range(3):
                    if acc:
                        block_starts.append(acc)
                    acc += 4 * (qc + 1)
            for b in block_starts:
                steps[b - 1], steps[b] = steps[b], steps[b - 1]

            # filler schedule: list of unit thunks per round, consumed one
            # per attention step (extras spill into later steps of the round)
            fillers = {
                0: [
                    lambda: qk_unit(1, 0), lambda: qk_unit(4, 0),
                    lambda: qk_unit(2, 0), lambda: v_unit(0),
                    lambda: v_unit(1), lambda: v_unit(2),
                    lambda: qk_unit(5, 0), lambda: v_unit(3),
                    lambda: qk_unit(0, 1), lambda: qk_unit(3, 1),
                    lambda: v_unit(4), lambda: v_unit(5),
                ],
                1: [
                    lambda: qk_unit(1, 1), lambda: qk_unit(4, 1),
                    lambda: qk_unit(2, 1), lambda: qk_unit(5, 1),
                    lambda: v_unit(6), lambda: v_unit(7),
                    lambda: qk_unit(0, 2), lambda: qk_unit(3, 2),
                    lambda: proj_unit(0, 0), lambda: proj_unit(0, 1),
                    lambda: qk_unit(1, 2), lambda: qk_unit(4, 2),
                    lambda: proj_unit(1, 0), lambda: proj_unit(1, 1),
                    lambda: v_unit(8), lambda: v_unit(9),
                    lambda: proj_unit(2, 0), lambda: proj_unit(2, 1),
                    lambda: qk_unit(2, 2), lambda: qk_unit(5, 2),
                    lambda: proj_unit(3, 0), lambda: proj_unit(3, 1),
                ],
                2: [
                    lambda: v_unit(10), lambda: v_unit(11),
                    lambda: qk_unit(0, 3), lambda: qk_unit(3, 3),
                    lambda: proj_unit(4, 0), lambda: proj_unit(4, 1),
                    lambda: qk_unit(1, 3), lambda: qk_unit(4, 3),
                    lambda: proj_unit(5, 0), lambda: proj_unit(5, 1),
                    lambda: qk_unit(2, 3), lambda: qk_unit(5, 3),
                    lambda: proj_unit(6, 0), lambda: proj_unit(6, 1),
                    lambda: v_unit(12), lambda: v_unit(13),
                    lambda: proj_unit(7, 0), lambda: proj_unit(7, 1),
                    lambda: v_unit(14), lambda: v_unit(15),
                ],
            }
            # spread fillers evenly across the round's steps
            round_first_step = {0: 0, 1: 12, 2: 36, 3: 72}
            round_len = {0: 12, 1: 24, 2: 36, 3: 48}
            # proj fillers for round r-1 must be emitted after the AV-lag
            # queue has popped norm(hp2, r-1): offset the spread by LAG+2.
            step_fillers = {}
            for r, units in fillers.items():
                n_steps = round_len[r]
                for i, u in enumerate(units):
                    s = round_first_step[r] + min(
                        i * n_steps // max(len(units), 1), n_steps - 1
                    )
                    step_fillers.setdefault(s, []).append(u)
            # round 3: place proj(8)/proj(9) right after the hp0/hp1 block
            # ends so the PE stays busy (HAM warm) through those norms;
            # proj(10)/proj(11) move to the epilogue to cover the last norm
            for i, (t_, h_) in enumerate([(8, 0), (8, 1), (10, 0), (10, 1)]):
                step_fillers.setdefault(72 + 16 + LAG + i, []).append(
                    lambda t_=t_, h_=h_: proj_unit(t_, h_)
                )
            for i, (t_, h_) in enumerate([(9, 0), (9, 1), (11, 0), (11, 1)]):
                step_fillers.setdefault(72 + 32 + LAG + i, []).append(
                    lambda t_=t_, h_=h_: proj_unit(t_, h_)
                )

            # prologue: just enough to unlock S(hp0, qc0); V chunks arrive as
            # step-0/1 fillers (AV lags by 3 steps)
            qk_unit(0, 0)
            qk_unit(3, 0)

            # pipeline loop
            pend = []  # (hp, qc, j, nj, sexp, av0, av1)
            block_avs = {}

            def emit_av(rec):
                hp, qc, j, nj, sexp, av0, av1 = rec
                av_pair(hp, qc, j, nj, sexp, av0, av1)
                if j == nj - 1:
                    norm(hp, qc, av0, av1)

            for idx, (hp, qc, j, nj) in enumerate(steps):
                if j == 0:
                    av0 = ps_av.tile([65, 512], F32, tag="av", name=f"av0_{hp}_{qc}")
                    av1 = ps_av.tile([65, 512], F32, tag="av", name=f"av1_{hp}_{qc}")
                    block_avs[(hp, qc)] = (av0, av1)
                av0, av1 = block_avs[(hp, qc)]
                psS = ps_s.tile([P, 1024], F32, tag="s", name="psS")
                s_pair(hp, qc, j, psS)
                sexp = sexp_pool.tile([P, 1024], DT, tag="sexp", name="sexp")
                exp_pair(hp, qc, j, psS, sexp)
                if j - 4 * qc >= 0:
                    mask_pair(hp, qc, j, sexp)
                pend.append((hp, qc, j, nj, sexp, av0, av1))
                if len(pend) > LAG:
                    emit_av(pend.pop(0))
                for u in step_fillers.get(idx, []):
                    u()
            while pend:
                emit_av(pend.pop(0))

            # epilogue: the last round's projection
            for t in range(12, 16):
                proj_unit(t, 0)
                proj_unit(t, 1)

    nc.finalize()
    return nc


def shard_inputs(x, w_attn, b_attn, w_proj, b_proj):
    """Host-side prep: slice per core, transpose x, cast to bf16."""
    scale = 1.0 / np.sqrt(D)
    tril = np.tril(np.ones((P, P), np.float32))
    # mask[k_local, q_local] = 1 where k <= q
    mask = tril.T.astype(BF16)
    in_maps = []
    for core in range(8):
        b, half = divmod(core, 2)
        h0 = half * NH
        cq = slice(h0 * D, (h0 + NH) * D)
        ck = slice(C + h0 * D, C + (h0 + NH) * D)
        cv = slice(2 * C + h0 * D, 2 * C + (h0 + NH) * D)
        wq = (w_attn[:, cq] * scale).astype(BF16)
        wk = w_attn[:, ck].astype(BF16)
        # pair-major: [C, 3 pairs, 256] with Q chunk then K chunk per pair
        wqk_pm = np.empty((C, 3, 2 * P), BF16)
        for p_ in range(3):
            wqk_pm[:, p_, 0:P] = wq[:, p_ * P : (p_ + 1) * P]
            wqk_pm[:, p_, P : 2 * P] = wk[:, p_ * P : (p_ + 1) * P]
        bqk = np.concatenate([(b_attn[cq] * scale), b_attn[ck]], axis=0).astype(
            np.float32
        )
        bqk_col = np.ascontiguousarray(bqk.reshape(2 * NH * D // P, P).T)
        wv = w_attn[:, cv].astype(BF16)
        bv = b_attn[cv].astype(BF16)[None, :]
        wp = w_proj[h0 * D : (h0 + NH) * D, :].astype(BF16)
        bp = (b_proj if half == 0 else np.zeros_like(b_proj)).astype(BF16)[None, :]
        xt = np.ascontiguousarray(x[b].T)  # [C, T] fp32
        # [P, QC, CCH, 512] t-block-major
        xt_tb = np.ascontiguousarray(
            xt.reshape(CCH, P, QC, 512).transpose(1, 2, 0, 3)
        )
        in_maps.append(
            {
                "xt": xt_tb.astype(BF16),
                "wqk": np.ascontiguousarray(
                    wqk_pm.reshape(CCH, P, 3, 2 * P).transpose(1, 2, 0, 3)
                ),
                "bqk": bqk_col,
                "wv": np.ascontiguousarray(
                    wv.reshape(CCH, P, NH * D).transpose(1, 0, 2)
                ),
                "bv": bv,
                "wp": np.ascontiguousarray(
                    wp.reshape(NH * D // P, P, C).transpose(1, 0, 2)
                ),
                "bp": bp,
                "mask": mask,
            }
        )
    return in_maps


_NC = None


def _get_nc():
    global _NC
    if _NC is None:
        _NC = build_nc()
    return _NC


def run_sharded(in_maps, trace=False, **kw):
    nc = _get_nc()
    return run_bass_kernel_spmd(nc, in_maps, core_ids=list(range(8)), trace=trace, **kw)


def gather(results):
    out = np.zeros((B, T, C), np.float32)
    for core in range(8):
        b = core // 2
        out[b] += results[core]["out"].astype(np.float32)
    return out


def kernel(x, w_attn, b_attn, w_proj, b_proj):
    x = np.asarray(x, np.float32)
    w_attn = np.asarray(w_attn, np.float32)
    b_attn = np.asarray(b_attn, np.float32)
    w_proj = np.asarray(w_proj, np.float32)
    b_proj = np.asarray(b_proj, np.float32)
    in_maps = shard_inputs(x, w_attn, b_attn, w_proj, b_proj)
    res = run_sharded(in_maps, trace=False)
    return gather(res.results)


# revision 30
# speedup vs baseline: 1.0278x; 1.0278x over previous
"""Causal self-attention (GPT-2 small block shape: B=4, T=2048, C=768, H=12, D=64)
on 8 TRN2 NeuronCores.

Sharding: core i handles batch b = i//2 and head-half = i%2 (6 heads each).
No cross-core collectives; the two half-head partial output projections per
batch are summed on the host during unshard (row-parallel c_proj).

Device kernel (per core, all matmuls bf16, fp32 PSUM accumulation), v2:
  - The two heads of a pair run as CONCURRENT row-tiled S matmuls (K=64 at
    PE row offsets 0/64) writing the two 512-col halves of one [128,1024]
    PSUM tile, so S-gen runs at full PE rate.
  - One exp per (pair, q-block, k-chunk) covers both heads; causally
    trimmed via a 3-D access pattern.  ScalarE(exp) and TensorE are the
    two co-critical engines (~125 us each); everything else hides.
  - Flattened software-pipelined emission: AV lags S by a few steps and
    QKV / output-projection matmuls are split into ~1 us filler units
    woven between attention steps (the Tile list scheduler uses program
    order as priority).
  - Softmax normalization: ones-column in V accumulates row sums into PSUM
    partition 64; reciprocal straight off PSUM, one partition-broadcast per
    pair, two DVE multiplies.
  - Output written bf16 (summed fp32 on host).
"""

import sys

if "/opt/trn_rl_repo" not in sys.path:
    sys.path.insert(0, "/opt/trn_rl_repo")

import numpy as np
import ml_dtypes

import concourse.bass as bass  # noqa: F401
import concourse.mybir as mybir
from concourse import bacc
from concourse.tile import TileContext
from concourse.bass_utils import run_bass_kernel_spmd

BF16 = ml_dtypes.bfloat16

B, T, C = 4, 2048, 768
H, D = 12, 64
NH = 6  # heads per core
P = 128
TC = T // P  # 16 t-chunks of 128
QC = T // 512  # 4 q-blocks of 512
CCH = C // P  # 6 contraction chunks

DT = mybir.dt.bfloat16
F32 = mybir.dt.float32
F8 = mybir.dt.float8e4
F8NP = ml_dtypes.float8_e4m3
EXP_SCALE = 1.0 / 8192.0  # undo the x32 fp8 pre-scales and 1/sqrt(D)


def build_nc():
    nc = bacc.Bacc()

    # t-block-major so each 512-col block is one per-partition-contiguous DMA
    xt_d = nc.declare_dram_parameter("xt", [P, QC, CCH, 512], DT, isOutput=False)
    # pair-major: wqk[p, pair, cc, 0:128]=Q chunk, [..., 128:256]=K chunk —
    # per-partition-contiguous per pair so each pair is one full-rate DMA
    wqk_d = nc.declare_dram_parameter("wqk", [P, 3, CCH, 2 * P], DT, isOutput=False)
    bqk_d = nc.declare_dram_parameter("bqk", [P, 2 * NH * D // P], F32, isOutput=False)
    wv_d = nc.declare_dram_parameter("wv", [P, CCH, NH * D], DT, isOutput=False)
    bv_d = nc.declare_dram_parameter("bv", [1, NH * D], DT, isOutput=False)
    wp_d = nc.declare_dram_parameter("wp", [P, NH * D // P, C], DT, isOutput=False)
    bp_d = nc.declare_dram_parameter("bp", [1, C], DT, isOutput=False)
    mask_d = nc.declare_dram_parameter("mask", [P, P], DT, isOutput=False)
    out_d = nc.declare_dram_parameter("out", [T, C], DT, isOutput=True)

    with TileContext(nc) as tc:
        with (
            tc.tile_pool(name="consts", bufs=1) as consts,
            tc.tile_pool(name="sexp", bufs=6) as sexp_pool,
            tc.tile_pool(name="inv", bufs=2) as inv_pool,
            tc.tile_pool(name="invb", bufs=2) as invb_pool,
            tc.tile_pool(name="outp", bufs=4) as outp,
            tc.tile_pool(name="ps_s", bufs=2, space="PSUM") as ps_s,
            tc.tile_pool(name="ps_av", bufs=2, space="PSUM") as ps_av,
            tc.tile_pool(name="ps_mm", bufs=2, space="PSUM") as ps_mm,
        ):
            # ---- input DMAs, one queue (a single InstDMACopy spreads over
            # all 16 SDMA engines; a second ring does not add HBM bandwidth),
            # issued on sync so the ACT queue stays clean for exps.
            # Ordered by first use.
            wqk_sb = consts.tile([P, 3, CCH, 2 * P], DT)
            nc.sync.dma_start(wqk_sb[:, 0], wqk_d[:, 0])
            bqk_sb = consts.tile([P, 2 * NH * D // P], F32)
            nc.sync.dma_start(bqk_sb[:], bqk_d[:])
            xt_sb = consts.tile([P, QC, CCH, 512], DT)
            nc.sync.dma_start(xt_sb[:, 0], xt_d[:, 0])
            nc.sync.dma_start(wqk_sb[:, 1], wqk_d[:, 1])
            nc.sync.dma_start(wqk_sb[:, 2], wqk_d[:, 2])
            mask_sb = consts.tile([P, P], DT)
            nc.sync.dma_start(mask_sb[:], mask_d[:])
            wv_sb = consts.tile([P, CCH, NH * D], DT)
            nc.sync.dma_start(wv_sb[:], wv_d[:])
            bv_sb = consts.tile([1, NH * D], DT)
            nc.sync.dma_start(bv_sb[:], bv_d[:])
            nc.sync.dma_start(xt_sb[:, 1], xt_d[:, 1])
            nc.sync.dma_start(xt_sb[:, 2], xt_d[:, 2])
            nc.sync.dma_start(xt_sb[:, 3], xt_d[:, 3])
            wp_sb = consts.tile([P, NH * D // P, C], DT)
            nc.sync.dma_start(wp_sb[:], wp_d[:])
            bp_sb = consts.tile([1, C], DT)
            nc.sync.dma_start(bp_sb[:], bp_d[:])

            # pre-warm the ACT exp table during the DMA-bound ramp
            warm = consts.tile([1, 8], F32)
            nc.gpsimd.memset(warm[:], 0.0)
            nc.scalar.activation(warm[:], warm[:], mybir.ActivationFunctionType.Exp)

            bvb = consts.tile([P, NH * D], DT)
            nc.gpsimd.partition_broadcast(bvb[:], bv_sb[:])
            bpb = consts.tile([P, C], DT)
            nc.gpsimd.partition_broadcast(bpb[:], bp_sb[:])

            # Q^T/K^T as head-pair tiles [128, T]: head 2p in partitions 0:64,
            # head 2p+1 in partitions 64:128.
            qtp = [consts.tile([P, T], DT, name=f"qtp{p}", tag=f"qtp{p}") for p in range(3)]
            ktp = [consts.tile([P, T], DT, name=f"ktp{p}", tag=f"ktp{p}") for p in range(3)]
            # V per t-chunk, heads side by side with a ones column: [128, 6, 65]
            vt = [consts.tile([P, NH, D + 1], DT, name=f"vt{t}", tag=f"vt{t}") for t in range(TC)]
            for t in range(TC):
                nc.gpsimd.memset(vt[t][:, :, D : D + 1], 1.0)
            # y^T per head-pair [128, T] bf16 (unnormalized until norm step)
            yt = [consts.tile([P, T], DT, name=f"yt{p}", tag=f"yt{p}") for p in range(3)]

            # ---- filler units (QKV projection / output projection) ----
            def qk_unit(fc, r):
                # feature chunk fc: 0..2 -> Q pair fc, 3..5 -> K pair fc-3
                pair, koff = (fc, 0) if fc < 3 else (fc - 3, P)
                pq = ps_mm.tile([P, 512], F32, tag="mm", name="pq")
                for cc in range(CCH):
                    nc.tensor.matmul(
                        pq[:],
                        wqk_sb[:, pair, cc, koff : koff + P],
                        xt_sb[:, r, cc, :],
                        start=(cc == 0),
                        stop=(cc == CCH - 1),
                        skip_group_check=True,
                    )
                dst = qtp[fc] if fc < 3 else ktp[fc - 3]
                nc.vector.tensor_scalar_add(
                    dst[:, r * 512 : (r + 1) * 512],
                    pq[:],
                    bqk_sb[:, fc : fc + 1],
                )

            def v_unit(t):
                pv = ps_mm.tile([P, NH * D], F32, tag="mm", name="pv")
                for cc in range(CCH):
                    nc.tensor.matmul(
                        pv[:],
                        xt_sb[:, t // 4, cc, (t % 4) * P : (t % 4 + 1) * P],
                        wv_sb[:, cc, :],
                        start=(cc == 0),
                        stop=(cc == CCH - 1),
                        skip_group_check=True,
                    )
                nc.vector.tensor_add(
                    vt[t][:, :, 0:D],
                    pv[:].rearrange("p (h d) -> p h d", d=D),
                    bvb[:].rearrange("p (h d) -> p h d", d=D),
                )

            proj_stg = {}

            def proj_unit(t, half):
                # halves share one [128, C] staging tile; half 1 sends the
                # whole row-block as a single contiguous DMA
                c0, c1 = (0, 384) if half == 0 else (384, C)
                pp = ps_mm.tile([P, c1 - c0], F32, tag="mm", name="pp")
                for cp in range(3):
                    nc.tensor.matmul(
                        pp[:],
                        yt[cp][:, t * P : (t + 1) * P],
                        wp_sb[:, cp, c0:c1],
                        start=(cp == 0),
                        stop=(cp == 2),
                        skip_group_check=True,
                    )
                if half == 0:
                    proj_stg[t] = outp.tile([P, C], DT, name="stg", tag="stg")
                stg = proj_stg[t]
                nc.vector.tensor_add(stg[:, c0:c1], pp[:], bpb[:, c0:c1])
                if half == 1:
                    nc.sync.dma_start(out_d[t * P : (t + 1) * P, :], stg[:])

            # ---- attention step pieces ----
            # per-block state: av tiles (h0, h1) and the step list
            def s_pair(hp, qc, j, psS):
                m = max(0, (j - 4 * qc) * P)
                js = slice(j * P, (j + 1) * P)
                qs = slice(qc * 512 + m, (qc + 1) * 512)
                nc.tensor.matmul(
                    psS[:, m:512],
                    ktp[hp][0:64, js],
                    qtp[hp][0:64, qs],
                    start=True,
                    stop=True,
                )
                nc.tensor.matmul(
                    psS[:, 512 + m : 1024],
                    ktp[hp][64:128, js],
                    qtp[hp][64:128, qs],
                    start=True,
                    stop=True,
                )

            def exp_pair(hp, qc, j, psS, sexp):
                m = max(0, (j - 4 * qc) * P)
                if m:
                    src = psS[:].rearrange("p (s q) -> p s q", s=2)[:, :, m:512]
                    dst = sexp[:].rearrange("p (s q) -> p s q", s=2)[:, :, m:512]
                else:
                    src, dst = psS[:], sexp[:]
                nc.scalar.activation(dst, src, mybir.ActivationFunctionType.Exp)

            def mask_pair(hp, qc, j, sexp):
                m = (j - 4 * qc) * P
                for s in (0, 1):
                    nc.vector.tensor_mul(
                        sexp[:, s * 512 + m : s * 512 + m + P],
                        sexp[:, s * 512 + m : s * 512 + m + P],
                        mask_sb[:],
                    )

            def av_pair(hp, qc, j, nj, sexp, av0, av1):
                m = max(0, (j - 4 * qc) * P)
                for s, av in ((0, av0), (1, av1)):
                    nc.tensor.matmul(
                        av[:, m:512],
                        vt[j][:, 2 * hp + s, :],
                        sexp[:, s * 512 + m : (s + 1) * 512],
                        start=(j == 0),
                        stop=(j == nj - 1),
                        skip_group_check=True,
                    )

            def norm(hp, qc, av0, av1):
                # reciprocal_approx_fast misreads PSUM at partition offset 64;
                # stage the sums rows through SBUF first (copies are cheap).
                sums = inv_pool.tile([1, 1024], F32, name="sums", tag="sums")
                nc.vector.tensor_copy(sums[:, 0:512], av0[64:65, :])
                nc.vector.tensor_copy(sums[:, 512:1024], av1[64:65, :])
                inv = inv_pool.tile([1, 1024], F32, name="inv", tag="inv")
                nc.vector.reciprocal_approx_fast(inv[:], sums[:])
                invb = invb_pool.tile([64, 1024], F32, name="invb")
                nc.gpsimd.partition_broadcast(invb[:], inv[:])
                qs = slice(qc * 512, (qc + 1) * 512)
                nc.vector.tensor_mul(yt[hp][0:64, qs], av0[0:64, :], invb[:, 0:512])
                nc.vector.tensor_mul(yt[hp][64:128, qs], av1[0:64, :], invb[:, 512:1024])

            # ---- flattened pipeline ----
            LAG = 4
            steps = []
            for qc in range(QC):
                for hp in range(3):
                    nj = 4 * (qc + 1)
                    for j in range(nj):
                        steps.append((hp, qc, j, nj))

            # filler schedule: list of unit thunks per round, consumed one
            # per attention step (extras spill into later steps of the round)
            fillers = {
                0: [
                    lambda: qk_unit(1, 0), lambda: qk_unit(4, 0),
                    lambda: qk_unit(2, 0), lambda: v_unit(0),
                    lambda: v_unit(1), lambda: v_unit(2),
                    lambda: v_unit(3), lambda: qk_unit(5, 0),
                    lambda: qk_unit(0, 1), lambda: qk_unit(3, 1),
                    lambda: v_unit(4), lambda: v_unit(5),
                ],
                1: [
                    lambda: qk_unit(1, 1), lambda: qk_unit(4, 1),
                    lambda: qk_unit(2, 1), lambda: qk_unit(5, 1),
                    lambda: v_unit(6), lambda: v_unit(7),
                    lambda: qk_unit(0, 2), lambda: qk_unit(3, 2),
                    lambda: proj_unit(0, 0), lambda: proj_unit(0, 1),
                    lambda: qk_unit(1, 2), lambda: qk_unit(4, 2),
                    lambda: proj_unit(1, 0), lambda: proj_unit(1, 1),
                    lambda: v_unit(8), lambda: v_unit(9),
                    lambda: proj_unit(2, 0), lambda: proj_unit(2, 1),
                    lambda: qk_unit(2, 2), lambda: qk_unit(5, 2),
                    lambda: proj_unit(3, 0), lambda: proj_unit(3, 1),
                ],
                2: [
                    lambda: v_unit(10), lambda: v_unit(11),
                    lambda: qk_unit(0, 3), lambda: qk_unit(3, 3),
                    lambda: proj_unit(4, 0), lambda: proj_unit(4, 1),
                    lambda: qk_unit(1, 3), lambda: qk_unit(4, 3),
                    lambda: proj_unit(5, 0), lambda: proj_unit(5, 1),
                    lambda: qk_unit(2, 3), lambda: qk_unit(5, 3),
                    lambda: proj_unit(6, 0), lambda: proj_unit(6, 1),
                    lambda: v_unit(12), lambda: v_unit(13),
                    lambda: proj_unit(7, 0), lambda: proj_unit(7, 1),
                    lambda: v_unit(14), lambda: v_unit(15),
                ],
            }
            # spread fillers evenly across the round's steps
            round_first_step = {0: 0, 1: 12, 2: 36, 3: 72}
            round_len = {0: 12, 1: 24, 2: 36, 3: 48}
            # proj fillers for round r-1 must be emitted after the AV-lag
            # queue has popped norm(hp2, r-1): offset the spread by LAG+2.
            step_fillers = {}
            for r, units in fillers.items():
                n_steps = round_len[r]
                for i, u in enumerate(units):
                    s = round_first_step[r] + min(
                        i * n_steps // max(len(units), 1), n_steps - 1
                    )
                    step_fillers.setdefault(s, []).append(u)
            # round 3: place proj(8)/proj(9) right after the hp0/hp1 block
            # ends so the PE stays busy (HAM warm) through those norms;
            # proj(10)/proj(11) move to the epilogue to cover the last norm
            for i, (t_, h_) in enumerate([(8, 0), (8, 1), (10, 0), (10, 1)]):
                step_fillers.setdefault(72 + 16 + LAG + i, []).append(
                    lambda t_=t_, h_=h_: proj_unit(t_, h_)
                )
            for i, (t_, h_) in enumerate([(9, 0), (9, 1), (11, 0), (11, 1)]):
                step_fillers.setdefault(72 + 32 + LAG + i, []).append(
                    lambda t_=t_, h_=h_: proj_unit(t_, h_)
                )

            # prologue: just enough to unlock S(hp0, qc0); V chunks arrive as
            # step-0/1 fillers (AV lags by 3 steps)
            qk_unit(0, 0)
            qk_unit(3, 0)

            # pipeline loop
            pend = []  # (hp, qc, j, nj, sexp, av0, av1)
            block_avs = {}

            def emit_av(rec):
                hp, qc, j, nj, sexp, av0, av1 = rec
                av_pair(hp, qc, j, nj, sexp, av0, av1)
                if j == nj - 1:
                    norm(hp, qc, av0, av1)

            for idx, (hp, qc, j, nj) in enumerate(steps):
                if j == 0:
                    av0 = ps_av.tile([65, 512], F32, tag="av", name=f"av0_{hp}_{qc}")
                    av1 = ps_av.tile([65, 512], F32, tag="av", name=f"av1_{hp}_{qc}")
                    block_avs[(hp, qc)] = (av0, av1)
                av0, av1 = block_avs[(hp, qc)]
                psS = ps_s.tile([P, 1024], F32, tag="s", name="psS")
                s_pair(hp, qc, j, psS)
                sexp = sexp_pool.tile([P, 1024], DT, tag="sexp", name="sexp")
                exp_pair(hp, qc, j, psS, sexp)
                if j - 4 * qc >= 0:
                    mask_pair(hp, qc, j, sexp)
                pend.append((hp, qc, j, nj, sexp, av0, av1))
                if len(pend) > LAG:
                    emit_av(pend.pop(0))
                for u in step_fillers.get(idx, []):
                    u()
            while pend:
                emit_av(pend.pop(0))

            # epilogue: the last round's projection
            for t in range(12, 16):
                proj_unit(t, 0)
                proj_unit(t, 1)

    nc.finalize()
    return nc


def shard_inputs(x, w_attn, b_attn, w_proj, b_proj):
    """Host-side prep: slice per core, transpose x, cast to bf16."""
    scale = 1.0 / np.sqrt(D)
    tril = np.tril(np.ones((P, P), np.float32))
    # mask[k_local, q_local] = 1 where k <= q
    mask = tril.T.astype(BF16)
    in_maps = []
    for core in range(8):
        b, half = divmod(core, 2)
        h0 = half * NH
        cq = slice(h0 * D, (h0 + NH) * D)
        ck = slice(C + h0 * D, C + (h0 + NH) * D)
        cv = slice(2 * C + h0 * D, 2 * C + (h0 + NH) * D)
        wq = (w_attn[:, cq] * scale).astype(BF16)
        wk = w_attn[:, ck].astype(BF16)
        # pair-major: [C, 3 pairs, 256] with Q chunk then K chunk per pair
        wqk_pm = np.empty((C, 3, 2 * P), BF16)
        for p_ in range(3):
            wqk_pm[:, p_, 0:P] = wq[:, p_ * P : (p_ + 1) * P]
            wqk_pm[:, p_, P : 2 * P] = wk[:, p_ * P : (p_ + 1) * P]
        bqk = np.concatenate([(b_attn[cq] * scale), b_attn[ck]], axis=0).astype(
            np.float32
        )
        bqk_col = np.ascontiguousarray(bqk.reshape(2 * NH * D // P, P).T)
        wv = w_attn[:, cv].astype(BF16)
        bv = b_attn[cv].astype(BF16)[None, :]
        wp = w_proj[h0 * D : (h0 + NH) * D, :].astype(BF16)
        bp = (b_proj if half == 0 else np.zeros_like(b_proj)).astype(BF16)[None, :]
        xt = np.ascontiguousarray(x[b].T)  # [C, T] fp32
        # [P, QC, CCH, 512] t-block-major
        xt_tb = np.ascontiguousarray(
            xt.reshape(CCH, P, QC, 512).transpose(1, 2, 0, 3)
        )
        in_maps.append(
            {
                "xt": xt_tb.astype(BF16),
                "wqk": np.ascontiguousarray(
                    wqk_pm.reshape(CCH, P, 3, 2 * P).transpose(1, 2, 0, 3)
                ),
                "bqk": bqk_col,
                "wv": np.ascontiguousarray(
                    wv.reshape(CCH, P, NH * D).transpose(1, 0, 2)
                ),
                "bv": bv,
                "wp": np.ascontiguousarray(
                    wp.reshape(NH * D // P, P, C).transpose(1, 0, 2)
                ),
                "bp": bp,
                "mask": mask,
            }
        )
    return in_maps


_NC = None


def _get_nc():
    global _NC
    if _NC is None:
        _NC = build_nc()
    return _NC


def run_sharded(in_maps, trace=False, **kw):
    nc = _get_nc()
    return run_bass_kernel_spmd(nc, in_maps, core_ids=list(range(8)), trace=trace, **kw)


def gather(results):
    out = np.zeros((B, T, C), np.float32)
    for core in range(8):
        b = core // 2
        out[b] += results[core]["out"].astype(np.float32)
    return out


def kernel(x, w_attn, b_attn, w_proj, b_proj):
    x = np.asarray(x, np.float32)
    w_attn = np.asarray(w_attn, np.float32)
    b_attn = np.asarray(b_attn, np.float32)
    w_proj = np.asarray(w_proj, np.float32)
    b_proj = np.asarray(b_proj, np.float32)
    in_maps = shard_inputs(x, w_attn, b_attn, w_proj, b_proj)
    res = run_sharded(in_maps, trace=False)
    return gather(res.results)


# revision 31
# speedup vs baseline: 1.0322x; 1.0043x over previous
"""Causal self-attention (GPT-2 small: B=4, T=2048, C=768, H=12, D=64)
on 8 TRN2 NeuronCores.

Sharding: core i handles batch b = i//2 and head-half = i%2 (6 heads each).
No cross-core collectives; the two half-head partial output projections per
batch are summed on the host during unshard (row-parallel c_proj).

Device kernel (per core, all matmuls bf16, fp32 PSUM accumulation):
  - The two heads of a pair run as CONCURRENT row-tiled S matmuls (K=64 at
    PE row offsets 0/64, tile_position auto-derived) writing the two
    512-col halves of one [128,1024] PSUM tile -> S-gen at full PE rate.
  - One exp per (pair, q-block, k-chunk) covers both heads, causally
    trimmed via a 3-D access pattern.  TensorE (~166 us) and ScalarE exp
    (~115 us) are the two critical engines; DVE/GpSimd/DMA hide.
  - Flattened software-pipelined emission: AV lags S by LAG steps; QKV and
    output-projection matmuls are split into ~1 us filler units woven
    between attention steps (the Tile list scheduler uses program order as
    priority).  Round-3 proj fillers sit right after block boundaries and
    the epilogue so the PE stays busy (HAM stays at full clock) through
    the final softmax normalizations.
  - Softmax normalization: ones-column in V accumulates row sums into PSUM
    partition 64; sums are copied to SBUF (custom-DVE reciprocal misreads
    PSUM at partition offset 64), reciprocal_approx_fast, one
    partition-broadcast per pair, two DVE multiplies into y^T.
  - DMA: all issues on the sync queue (keeping the ACT queue free of
    head-of-line DMA waits); inputs are laid out host-side so every
    transfer is per-partition contiguous at full HBM rate, ordered by
    first use; output is written bf16 (summed in fp32 on the host).
"""

import sys

if "/opt/trn_rl_repo" not in sys.path:
    sys.path.insert(0, "/opt/trn_rl_repo")

import numpy as np
import ml_dtypes

import concourse.bass as bass  # noqa: F401
import concourse.mybir as mybir
from concourse import bacc
from concourse.tile import TileContext
from concourse.bass_utils import run_bass_kernel_spmd

BF16 = ml_dtypes.bfloat16

B, T, C = 4, 2048, 768
H, D = 12, 64
NH = 6  # heads per core
P = 128
TC = T // P  # 16 t-chunks of 128
QC = T // 512  # 4 q-blocks of 512
CCH = C // P  # 6 contraction chunks

DT = mybir.dt.bfloat16
F32 = mybir.dt.float32


def build_nc():
    nc = bacc.Bacc()

    # t-block-major so each 512-col block is one per-partition-contiguous DMA
    xt_d = nc.declare_dram_parameter("xt", [P, QC, CCH, 512], DT, isOutput=False)
    # pair-major: wqk[p, pair, cc, 0:128]=Q chunk, [..., 128:256]=K chunk —
    # per-partition-contiguous per pair so each pair is one full-rate DMA
    wqk_d = nc.declare_dram_parameter("wqk", [P, 3, CCH, 2 * P], DT, isOutput=False)
    bqk_d = nc.declare_dram_parameter("bqk", [P, 2 * NH * D // P], F32, isOutput=False)
    wv_d = nc.declare_dram_parameter("wv", [P, CCH, NH * D], DT, isOutput=False)
    bv_d = nc.declare_dram_parameter("bv", [1, NH * D], DT, isOutput=False)
    wp_d = nc.declare_dram_parameter("wp", [P, NH * D // P, C], DT, isOutput=False)
    bp_d = nc.declare_dram_parameter("bp", [1, C], DT, isOutput=False)
    mask_d = nc.declare_dram_parameter("mask", [P, P], DT, isOutput=False)
    out_d = nc.declare_dram_parameter("out", [T, C], DT, isOutput=True)

    with TileContext(nc) as tc:
        with (
            tc.tile_pool(name="consts", bufs=1) as consts,
            tc.tile_pool(name="sexp", bufs=6) as sexp_pool,
            tc.tile_pool(name="inv", bufs=2) as inv_pool,
            tc.tile_pool(name="invb", bufs=2) as invb_pool,
            tc.tile_pool(name="outp", bufs=4) as outp,
            tc.tile_pool(name="ps_s", bufs=2, space="PSUM") as ps_s,
            tc.tile_pool(name="ps_av", bufs=2, space="PSUM") as ps_av,
            tc.tile_pool(name="ps_mm", bufs=2, space="PSUM") as ps_mm,
        ):
            # ---- input DMAs, one queue (a single InstDMACopy spreads over
            # all 16 SDMA engines; a second ring does not add HBM bandwidth),
            # issued on sync so the ACT queue stays clean for exps.
            # Ordered by first use.
            wqk_sb = consts.tile([P, 3, CCH, 2 * P], DT)
            nc.sync.dma_start(wqk_sb[:, 0], wqk_d[:, 0])
            bqk_sb = consts.tile([P, 2 * NH * D // P], F32)
            nc.sync.dma_start(bqk_sb[:], bqk_d[:])
            xt_sb = consts.tile([P, QC, CCH, 512], DT)
            nc.sync.dma_start(xt_sb[:, 0], xt_d[:, 0])
            nc.sync.dma_start(wqk_sb[:, 1], wqk_d[:, 1])
            nc.sync.dma_start(wqk_sb[:, 2], wqk_d[:, 2])
            mask_sb = consts.tile([P, P], DT)
            nc.sync.dma_start(mask_sb[:], mask_d[:])
            wv_sb = consts.tile([P, CCH, NH * D], DT)
            nc.sync.dma_start(wv_sb[:], wv_d[:])
            bv_sb = consts.tile([1, NH * D], DT)
            nc.sync.dma_start(bv_sb[:], bv_d[:])
            nc.sync.dma_start(xt_sb[:, 1], xt_d[:, 1])
            nc.sync.dma_start(xt_sb[:, 2], xt_d[:, 2])
            nc.sync.dma_start(xt_sb[:, 3], xt_d[:, 3])
            wp_sb = consts.tile([P, NH * D // P, C], DT)
            nc.sync.dma_start(wp_sb[:], wp_d[:])
            bp_sb = consts.tile([1, C], DT)
            nc.sync.dma_start(bp_sb[:], bp_d[:])

            # pre-warm the ACT exp table during the DMA-bound ramp
            warm = consts.tile([1, 8], F32)
            nc.gpsimd.memset(warm[:], 0.0)
            nc.scalar.activation(warm[:], warm[:], mybir.ActivationFunctionType.Exp)

            bvb = consts.tile([P, NH * D], DT)
            nc.gpsimd.partition_broadcast(bvb[:], bv_sb[:])
            bpb = consts.tile([P, C], DT)
            nc.gpsimd.partition_broadcast(bpb[:], bp_sb[:])

            # Q^T/K^T as head-pair tiles [128, T]: head 2p in partitions 0:64,
            # head 2p+1 in partitions 64:128.
            qtp = [consts.tile([P, T], DT, name=f"qtp{p}", tag=f"qtp{p}") for p in range(3)]
            ktp = [consts.tile([P, T], DT, name=f"ktp{p}", tag=f"ktp{p}") for p in range(3)]
            # V per t-chunk, heads side by side with a ones column: [128, 6, 65]
            vt = [consts.tile([P, NH, D + 1], DT, name=f"vt{t}", tag=f"vt{t}") for t in range(TC)]
            for t in range(TC):
                nc.gpsimd.memset(vt[t][:, :, D : D + 1], 1.0)
            # y^T per head-pair [128, T] bf16 (unnormalized until norm step)
            yt = [consts.tile([P, T], DT, name=f"yt{p}", tag=f"yt{p}") for p in range(3)]

            # ---- filler units (QKV projection / output projection) ----
            def qk_unit(fc, r):
                # feature chunk fc: 0..2 -> Q pair fc, 3..5 -> K pair fc-3
                pair, koff = (fc, 0) if fc < 3 else (fc - 3, P)
                pq = ps_mm.tile([P, 512], F32, tag="mm", name="pq")
                for cc in range(CCH):
                    nc.tensor.matmul(
                        pq[:],
                        wqk_sb[:, pair, cc, koff : koff + P],
                        xt_sb[:, r, cc, :],
                        start=(cc == 0),
                        stop=(cc == CCH - 1),
                        skip_group_check=True,
                    )
                dst = qtp[fc] if fc < 3 else ktp[fc - 3]
                nc.vector.tensor_scalar_add(
                    dst[:, r * 512 : (r + 1) * 512],
                    pq[:],
                    bqk_sb[:, fc : fc + 1],
                )

            def v_unit(t):
                pv = ps_mm.tile([P, NH * D], F32, tag="mm", name="pv")
                for cc in range(CCH):
                    nc.tensor.matmul(
                        pv[:],
                        xt_sb[:, t // 4, cc, (t % 4) * P : (t % 4 + 1) * P],
                        wv_sb[:, cc, :],
                        start=(cc == 0),
                        stop=(cc == CCH - 1),
                        skip_group_check=True,
                    )
                nc.vector.tensor_add(
                    vt[t][:, :, 0:D],
                    pv[:].rearrange("p (h d) -> p h d", d=D),
                    bvb[:].rearrange("p (h d) -> p h d", d=D),
                )

            proj_stg = {}

            def proj_unit(t, half):
                # halves share one [128, C] staging tile; half 1 sends the
                # whole row-block as a single contiguous DMA
                c0, c1 = (0, 384) if half == 0 else (384, C)
                pp = ps_mm.tile([P, c1 - c0], F32, tag="mm", name="pp")
                for cp in range(3):
                    nc.tensor.matmul(
                        pp[:],
                        yt[cp][:, t * P : (t + 1) * P],
                        wp_sb[:, cp, c0:c1],
                        start=(cp == 0),
                        stop=(cp == 2),
                        skip_group_check=True,
                    )
                if half == 0:
                    proj_stg[t] = outp.tile([P, C], DT, name="stg", tag="stg")
                stg = proj_stg[t]
                nc.vector.tensor_add(stg[:, c0:c1], pp[:], bpb[:, c0:c1])
                if half == 1:
                    nc.sync.dma_start(out_d[t * P : (t + 1) * P, :], stg[:])

            # ---- attention step pieces ----
            # per-block state: av tiles (h0, h1) and the step list
            def s_pair(hp, qc, j, psS):
                m = max(0, (j - 4 * qc) * P)
                js = slice(j * P, (j + 1) * P)
                qs = slice(qc * 512 + m, (qc + 1) * 512)
                nc.tensor.matmul(
                    psS[:, m:512],
                    ktp[hp][0:64, js],
                    qtp[hp][0:64, qs],
                    start=True,
                    stop=True,
                )
                nc.tensor.matmul(
                    psS[:, 512 + m : 1024],
                    ktp[hp][64:128, js],
                    qtp[hp][64:128, qs],
                    start=True,
                    stop=True,
                )

            def exp_pair(hp, qc, j, psS, sexp):
                m = max(0, (j - 4 * qc) * P)
                if m:
                    src = psS[:].rearrange("p (s q) -> p s q", s=2)[:, :, m:512]
                    dst = sexp[:].rearrange("p (s q) -> p s q", s=2)[:, :, m:512]
                else:
                    src, dst = psS[:], sexp[:]
                nc.scalar.activation(dst, src, mybir.ActivationFunctionType.Exp)

            def mask_pair(hp, qc, j, sexp):
                m = (j - 4 * qc) * P
                for s in (0, 1):
                    nc.vector.tensor_mul(
                        sexp[:, s * 512 + m : s * 512 + m + P],
                        sexp[:, s * 512 + m : s * 512 + m + P],
                        mask_sb[:],
                    )

            def av_pair(hp, qc, j, nj, sexp, av0, av1):
                m = max(0, (j - 4 * qc) * P)
                for s, av in ((0, av0), (1, av1)):
                    nc.tensor.matmul(
                        av[:, m:512],
                        vt[j][:, 2 * hp + s, :],
                        sexp[:, s * 512 + m : (s + 1) * 512],
                        start=(j == 0),
                        stop=(j == nj - 1),
                        skip_group_check=True,
                    )

            def norm(hp, qc, av0, av1):
                # reciprocal_approx_fast misreads PSUM at partition offset 64;
                # stage the sums rows through SBUF first (copies are cheap).
                sums = inv_pool.tile([1, 1024], F32, name="sums", tag="sums")
                nc.vector.tensor_copy(sums[:, 0:512], av0[64:65, :])
                nc.vector.tensor_copy(sums[:, 512:1024], av1[64:65, :])
                inv = inv_pool.tile([1, 1024], F32, name="inv", tag="inv")
                nc.vector.reciprocal_approx_fast(inv[:], sums[:])
                invb = invb_pool.tile([64, 1024], F32, name="invb")
                nc.gpsimd.partition_broadcast(invb[:], inv[:])
                qs = slice(qc * 512, (qc + 1) * 512)
                nc.vector.tensor_mul(yt[hp][0:64, qs], av0[0:64, :], invb[:, 0:512])
                nc.vector.tensor_mul(yt[hp][64:128, qs], av1[0:64, :], invb[:, 512:1024])

            # ---- flattened pipeline ----
            LAG = 4
            steps = []
            for qc in range(QC):
                for hp in range(3):
                    nj = 4 * (qc + 1)
                    for j in range(nj):
                        steps.append((hp, qc, j, nj))

            # filler schedule: list of unit thunks per round, consumed one
            # per attention step (extras spill into later steps of the round)
            fillers = {
                0: [
                    lambda: qk_unit(1, 0), lambda: qk_unit(4, 0),
                    lambda: qk_unit(2, 0), lambda: v_unit(0),
                    lambda: v_unit(1), lambda: v_unit(2),
                    lambda: v_unit(3), lambda: qk_unit(5, 0),
                    lambda: qk_unit(0, 1), lambda: qk_unit(3, 1),
                    lambda: v_unit(4), lambda: v_unit(5),
                ],
                1: [
                    lambda: qk_unit(1, 1), lambda: qk_unit(4, 1),
                    lambda: qk_unit(2, 1), lambda: qk_unit(5, 1),
                    lambda: v_unit(6), lambda: v_unit(7),
                    lambda: qk_unit(0, 2), lambda: qk_unit(3, 2),
                    lambda: proj_unit(0, 0), lambda: proj_unit(0, 1),
                    lambda: qk_unit(1, 2), lambda: qk_unit(4, 2),
                    lambda: proj_unit(1, 0), lambda: proj_unit(1, 1),
                    lambda: v_unit(8), lambda: v_unit(9),
                    lambda: proj_unit(2, 0), lambda: proj_unit(2, 1),
                    lambda: qk_unit(2, 2), lambda: qk_unit(5, 2),
                    lambda: proj_unit(3, 0), lambda: proj_unit(3, 1),
                ],
                2: [
                    lambda: v_unit(10), lambda: v_unit(11),
                    lambda: qk_unit(0, 3), lambda: qk_unit(3, 3),
                    lambda: proj_unit(4, 0), lambda: proj_unit(4, 1),
                    lambda: qk_unit(1, 3), lambda: qk_unit(4, 3),
                    lambda: proj_unit(5, 0), lambda: proj_unit(5, 1),
                    lambda: qk_unit(2, 3), lambda: qk_unit(5, 3),
                    lambda: proj_unit(6, 0), lambda: proj_unit(6, 1),
                    lambda: v_unit(12), lambda: v_unit(13),
                    lambda: proj_unit(7, 0), lambda: proj_unit(7, 1),
                    lambda: v_unit(14), lambda: v_unit(15),
                ],
            }
            # spread fillers evenly across the round's steps
            round_first_step = {0: 0, 1: 12, 2: 36, 3: 72}
            round_len = {0: 12, 1: 24, 2: 36, 3: 48}
            # proj fillers for round r-1 must be emitted after the AV-lag
            # queue has popped norm(hp2, r-1): offset the spread by LAG+2.
            step_fillers = {}
            for r, units in fillers.items():
                n_steps = round_len[r]
                for i, u in enumerate(units):
                    s = round_first_step[r] + min(
                        i * n_steps // max(len(units), 1), n_steps - 1
                    )
                    step_fillers.setdefault(s, []).append(u)
            # round 3: place proj(8)/proj(9) right after the hp0/hp1 block
            # ends so the PE stays busy (HAM warm) through those norms;
            # proj(10)/proj(11) move to the epilogue to cover the last norm
            for i, (t_, h_) in enumerate([(8, 0), (8, 1), (10, 0), (10, 1)]):
                step_fillers.setdefault(72 + 16 + LAG + i, []).append(
                    lambda t_=t_, h_=h_: proj_unit(t_, h_)
                )
            for i, (t_, h_) in enumerate([(9, 0), (9, 1), (11, 0), (11, 1)]):
                step_fillers.setdefault(72 + 32 + LAG + i, []).append(
                    lambda t_=t_, h_=h_: proj_unit(t_, h_)
                )

            # prologue: just enough to unlock S(hp0, qc0); V chunks arrive as
            # step-0/1 fillers (AV lags by 3 steps)
            qk_unit(0, 0)
            qk_unit(3, 0)

            # pipeline loop
            pend = []  # (hp, qc, j, nj, sexp, av0, av1)
            block_avs = {}

            def emit_av(rec):
                hp, qc, j, nj, sexp, av0, av1 = rec
                av_pair(hp, qc, j, nj, sexp, av0, av1)
                if j == nj - 1:
                    norm(hp, qc, av0, av1)

            for idx, (hp, qc, j, nj) in enumerate(steps):
                if j == 0:
                    av0 = ps_av.tile([65, 512], F32, tag="av", name=f"av0_{hp}_{qc}")
                    av1 = ps_av.tile([65, 512], F32, tag="av", name=f"av1_{hp}_{qc}")
                    block_avs[(hp, qc)] = (av0, av1)
                av0, av1 = block_avs[(hp, qc)]
                psS = ps_s.tile([P, 1024], F32, tag="s", name="psS")
                s_pair(hp, qc, j, psS)
                sexp = sexp_pool.tile([P, 1024], DT, tag="sexp", name="sexp")
                exp_pair(hp, qc, j, psS, sexp)
                if j - 4 * qc >= 0:
                    mask_pair(hp, qc, j, sexp)
                pend.append((hp, qc, j, nj, sexp, av0, av1))
                if len(pend) > LAG:
                    emit_av(pend.pop(0))
                for u in step_fillers.get(idx, []):
                    u()
            while pend:
                emit_av(pend.pop(0))

            # epilogue: the last round's projection
            for t in range(12, 16):
                proj_unit(t, 0)
                proj_unit(t, 1)

    nc.finalize()
    return nc


def shard_inputs(x, w_attn, b_attn, w_proj, b_proj):
    """Host-side prep: slice per core, transpose x, cast to bf16."""
    scale = 1.0 / np.sqrt(D)
    tril = np.tril(np.ones((P, P), np.float32))
    # mask[k_local, q_local] = 1 where k <= q
    mask = tril.T.astype(BF16)
    in_maps = []
    for core in range(8):
        b, half = divmod(core, 2)
        h0 = half * NH
        cq = slice(h0 * D, (h0 + NH) * D)
        ck = slice(C + h0 * D, C + (h0 + NH) * D)
        cv = slice(2 * C + h0 * D, 2 * C + (h0 + NH) * D)
        wq = (w_attn[:, cq] * scale).astype(BF16)
        wk = w_attn[:, ck].astype(BF16)
        # pair-major: [C, 3 pairs, 256] with Q chunk then K chunk per pair
        wqk_pm = np.empty((C, 3, 2 * P), BF16)
        for p_ in range(3):
            wqk_pm[:, p_, 0:P] = wq[:, p_ * P : (p_ + 1) * P]
            wqk_pm[:, p_, P : 2 * P] = wk[:, p_ * P : (p_ + 1) * P]
        bqk = np.concatenate([(b_attn[cq] * scale), b_attn[ck]], axis=0).astype(
            np.float32
        )
        bqk_col = np.ascontiguousarray(bqk.reshape(2 * NH * D // P, P).T)
        wv = w_attn[:, cv].astype(BF16)
        bv = b_attn[cv].astype(BF16)[None, :]
        wp = w_proj[h0 * D : (h0 + NH) * D, :].astype(BF16)
        bp = (b_proj if half == 0 else np.zeros_like(b_proj)).astype(BF16)[None, :]
        xt = np.ascontiguousarray(x[b].T)  # [C, T] fp32
        # [P, QC, CCH, 512] t-block-major
        xt_tb = np.ascontiguousarray(
            xt.reshape(CCH, P, QC, 512).transpose(1, 2, 0, 3)
        )
        in_maps.append(
            {
                "xt": xt_tb.astype(BF16),
                "wqk": np.ascontiguousarray(
                    wqk_pm.reshape(CCH, P, 3, 2 * P).transpose(1, 2, 0, 3)
                ),
                "bqk": bqk_col,
                "wv": np.ascontiguousarray(
                    wv.reshape(CCH, P, NH * D).transpose(1, 0, 2)
                ),
                "bv": bv,
                "wp": np.ascontiguousarray(
                    wp.reshape(NH * D // P, P, C).transpose(1, 0, 2)
                ),
                "bp": bp,
                "mask": mask,
            }
        )
    return in_maps


_NC = None


def _get_nc():
    global _NC
    if _NC is None:
        _NC = build_nc()
    return _NC


def run_sharded(in_maps, trace=False, **kw):
    nc = _get_nc()
    return run_bass_kernel_spmd(nc, in_maps, core_ids=list(range(8)), trace=trace, **kw)


def gather(results):
    out = np.zeros((B, T, C), np.float32)
    for core in range(8):
        b = core // 2
        out[b] += results[core]["out"].astype(np.float32)
    return out


def kernel(x, w_attn, b_attn, w_proj, b_proj):
    x = np.asarray(x, np.float32)
    w_attn = np.asarray(w_attn, np.float32)
    b_attn = np.asarray(b_attn, np.float32)
    w_proj = np.asarray(w_proj, np.float32)
    b_proj = np.asarray(b_proj, np.float32)
    in_maps = shard_inputs(x, w_attn, b_attn, w_proj, b_proj)
    res = run_sharded(in_maps, trace=False)
    return gather(res.results)


# revision 32
# speedup vs baseline: 1.0769x; 1.0433x over previous
"""Causal self-attention (GPT-2 small: B=4, T=2048, C=768, H=12, D=64)
on 8 TRN2 NeuronCores.

Sharding: core i handles batch b = i//2 and head-half = i%2 (6 heads each).
No cross-core collectives; the two half-head partial output projections per
batch are summed on the host during unshard (row-parallel c_proj).

Device kernel (per core, all matmuls bf16, fp32 PSUM accumulation):
  - The two heads of a pair run as CONCURRENT row-tiled S matmuls (K=64 at
    PE row offsets 0/64, tile_position auto-derived) writing the two
    512-col halves of one [128,1024] PSUM tile -> S-gen at full PE rate.
  - One exp per (pair, q-block, k-chunk) covers both heads, causally
    trimmed via a 3-D access pattern.  TensorE (~166 us) and ScalarE exp
    (~115 us) are the two critical engines; DVE/GpSimd/DMA hide.
  - Flattened software-pipelined emission: AV lags S by LAG steps; QKV and
    output-projection matmuls are split into ~1 us filler units woven
    between attention steps (the Tile list scheduler uses program order as
    priority).  Round-3 proj fillers sit right after block boundaries and
    the epilogue so the PE stays busy (HAM stays at full clock) through
    the final softmax normalizations.
  - Softmax normalization: ones-column in V accumulates row sums into PSUM
    partition 64; sums are copied to SBUF (custom-DVE reciprocal misreads
    PSUM at partition offset 64), reciprocal_approx_fast, one
    partition-broadcast per pair, two DVE multiplies into y^T.
  - DMA: all issues on the sync queue (keeping the ACT queue free of
    head-of-line DMA waits); inputs are laid out host-side so every
    transfer is per-partition contiguous at full HBM rate, ordered by
    first use; output is written bf16 (summed in fp32 on the host).
"""

import sys

if "/opt/trn_rl_repo" not in sys.path:
    sys.path.insert(0, "/opt/trn_rl_repo")

import numpy as np
import ml_dtypes

import concourse.bass as bass  # noqa: F401
import concourse.mybir as mybir
from concourse import bacc
from concourse.tile import TileContext
from concourse.bass_utils import run_bass_kernel_spmd

BF16 = ml_dtypes.bfloat16

B, T, C = 4, 2048, 768
H, D = 12, 64
NH = 6  # heads per core
P = 128
TC = T // P  # 16 t-chunks of 128
QC = T // 512  # 4 q-blocks of 512
CCH = C // P  # 6 contraction chunks

DT = mybir.dt.bfloat16
F32 = mybir.dt.float32


def build_nc():
    nc = bacc.Bacc()

    # t-block-major so each 512-col block is one per-partition-contiguous DMA
    xt_d = nc.declare_dram_parameter("xt", [P, QC, CCH, 512], DT, isOutput=False)
    # pair-major: wqk[p, pair, cc, 0:128]=Q chunk, [..., 128:256]=K chunk —
    # per-partition-contiguous per pair so each pair is one full-rate DMA
    wqk_d = nc.declare_dram_parameter("wqk", [P, 3, CCH, 2 * P], DT, isOutput=False)
    bqk_d = nc.declare_dram_parameter("bqk", [P, 2 * NH * D // P], F32, isOutput=False)
    wv_d = nc.declare_dram_parameter("wv", [P, CCH, NH * D], DT, isOutput=False)
    bv_d = nc.declare_dram_parameter("bv", [1, NH * D], DT, isOutput=False)
    wp_d = nc.declare_dram_parameter("wp", [P, NH * D // P, C], DT, isOutput=False)
    bp_d = nc.declare_dram_parameter("bp", [1, C], DT, isOutput=False)
    mask_d = nc.declare_dram_parameter("mask", [P, P], DT, isOutput=False)
    out_d = nc.declare_dram_parameter("out", [T, C], DT, isOutput=True)

    with TileContext(nc) as tc:
        with (
            tc.tile_pool(name="consts", bufs=1) as consts,
            tc.tile_pool(name="sexp", bufs=6) as sexp_pool,
            tc.tile_pool(name="inv", bufs=2) as inv_pool,
            tc.tile_pool(name="invb", bufs=2) as invb_pool,
            tc.tile_pool(name="outp", bufs=4) as outp,
            tc.tile_pool(name="ps_s", bufs=2, space="PSUM") as ps_s,
            tc.tile_pool(name="ps_av", bufs=2, space="PSUM") as ps_av,
            tc.tile_pool(name="ps_mm", bufs=2, space="PSUM") as ps_mm,
        ):
            # ---- input DMAs, one queue (a single InstDMACopy spreads over
            # all 16 SDMA engines; a second ring does not add HBM bandwidth),
            # issued on sync so the ACT queue stays clean for exps.
            # Ordered by first use.
            wqk_sb = consts.tile([P, 3, CCH, 2 * P], DT)
            nc.sync.dma_start(wqk_sb[:, 0], wqk_d[:, 0])
            bqk_sb = consts.tile([P, 2 * NH * D // P], F32)
            nc.sync.dma_start(bqk_sb[:], bqk_d[:])
            xt_sb = consts.tile([P, QC, CCH, 512], DT)
            nc.sync.dma_start(xt_sb[:, 0], xt_d[:, 0])
            nc.sync.dma_start(wqk_sb[:, 1], wqk_d[:, 1])
            nc.sync.dma_start(wqk_sb[:, 2], wqk_d[:, 2])
            mask_sb = consts.tile([P, P], DT)
            nc.sync.dma_start(mask_sb[:], mask_d[:])
            wv_sb = consts.tile([P, CCH, NH * D], DT)
            nc.sync.dma_start(wv_sb[:], wv_d[:])
            bv_sb = consts.tile([1, NH * D], DT)
            nc.sync.dma_start(bv_sb[:], bv_d[:])
            nc.sync.dma_start(xt_sb[:, 1], xt_d[:, 1])
            nc.sync.dma_start(xt_sb[:, 2], xt_d[:, 2])
            nc.sync.dma_start(xt_sb[:, 3], xt_d[:, 3])
            wp_sb = consts.tile([P, NH * D // P, C], DT)
            nc.sync.dma_start(wp_sb[:], wp_d[:])
            bp_sb = consts.tile([1, C], DT)
            nc.sync.dma_start(bp_sb[:], bp_d[:])

            # pre-warm the ACT exp table during the DMA-bound ramp
            warm = consts.tile([1, 8], F32)
            nc.gpsimd.memset(warm[:], 0.0)
            nc.scalar.activation(warm[:], warm[:], mybir.ActivationFunctionType.Exp)

            bvb = consts.tile([P, NH * D], DT)
            nc.gpsimd.partition_broadcast(bvb[:], bv_sb[:])
            bpb = consts.tile([P, C], DT)
            nc.gpsimd.partition_broadcast(bpb[:], bp_sb[:])

            # Q^T/K^T as head-pair tiles [128, T]: head 2p in partitions 0:64,
            # head 2p+1 in partitions 64:128.
            qtp = [consts.tile([P, T], DT, name=f"qtp{p}", tag=f"qtp{p}") for p in range(3)]
            ktp = [consts.tile([P, T], DT, name=f"ktp{p}", tag=f"ktp{p}") for p in range(3)]
            # V per t-chunk, heads side by side with a ones column: [128, 6, 65]
            vt = [consts.tile([P, NH, D + 1], DT, name=f"vt{t}", tag=f"vt{t}") for t in range(TC)]
            for t in range(TC):
                nc.gpsimd.memset(vt[t][:, :, D : D + 1], 1.0)
            # y^T per head-pair [128, T] bf16 (unnormalized until norm step)
            yt = [consts.tile([P, T], DT, name=f"yt{p}", tag=f"yt{p}") for p in range(3)]

            # ---- filler units (QKV projection / output projection) ----
            def qk_unit(fc, r):
                # feature chunk fc: 0..2 -> Q pair fc, 3..5 -> K pair fc-3
                pair, koff = (fc, 0) if fc < 3 else (fc - 3, P)
                pq = ps_mm.tile([P, 512], F32, tag="mm", name="pq")
                for cc in range(CCH):
                    nc.tensor.matmul(
                        pq[:],
                        wqk_sb[:, pair, cc, koff : koff + P],
                        xt_sb[:, r, cc, :],
                        start=(cc == 0),
                        stop=(cc == CCH - 1),
                        skip_group_check=True,
                    )
                dst = qtp[fc] if fc < 3 else ktp[fc - 3]
                nc.vector.tensor_scalar_add(
                    dst[:, r * 512 : (r + 1) * 512],
                    pq[:],
                    bqk_sb[:, fc : fc + 1],
                )

            def v_unit(t):
                pv = ps_mm.tile([P, NH * D], F32, tag="mm", name="pv")
                for cc in range(CCH):
                    nc.tensor.matmul(
                        pv[:],
                        xt_sb[:, t // 4, cc, (t % 4) * P : (t % 4 + 1) * P],
                        wv_sb[:, cc, :],
                        start=(cc == 0),
                        stop=(cc == CCH - 1),
                        skip_group_check=True,
                    )
                nc.vector.tensor_add(
                    vt[t][:, :, 0:D],
                    pv[:].rearrange("p (h d) -> p h d", d=D),
                    bvb[:].rearrange("p (h d) -> p h d", d=D),
                )

            proj_stg = {}

            def proj_unit(t, half):
                # halves share one [128, C] staging tile; half 1 sends the
                # whole row-block as a single contiguous DMA
                c0, c1 = (0, 384) if half == 0 else (384, C)
                pp = ps_mm.tile([P, c1 - c0], F32, tag="mm", name="pp")
                for cp in range(3):
                    nc.tensor.matmul(
                        pp[:],
                        yt[cp][:, t * P : (t + 1) * P],
                        wp_sb[:, cp, c0:c1],
                        start=(cp == 0),
                        stop=(cp == 2),
                        skip_group_check=True,
                    )
                if half == 0:
                    proj_stg[t] = outp.tile([P, C], DT, name="stg", tag="stg")
                stg = proj_stg[t]
                nc.vector.tensor_add(stg[:, c0:c1], pp[:], bpb[:, c0:c1])
                if half == 1:
                    nc.sync.dma_start(out_d[t * P : (t + 1) * P, :], stg[:])

            # ---- attention step pieces ----
            # per-block state: av tiles (h0, h1) and the step list
            def s_pair(hp, qc, j, psS):
                m = max(0, (j - 4 * qc) * P)
                js = slice(j * P, (j + 1) * P)
                qs = slice(qc * 512 + m, (qc + 1) * 512)
                nc.tensor.matmul(
                    psS[:, m:512],
                    ktp[hp][0:64, js],
                    qtp[hp][0:64, qs],
                    start=True,
                    stop=True,
                )
                nc.tensor.matmul(
                    psS[:, 512 + m : 1024],
                    ktp[hp][64:128, js],
                    qtp[hp][64:128, qs],
                    start=True,
                    stop=True,
                )

            def exp_pair(hp, qc, j, psS, sexp):
                m = max(0, (j - 4 * qc) * P)
                if m:
                    src = psS[:].rearrange("p (s q) -> p s q", s=2)[:, :, m:512]
                    dst = sexp[:].rearrange("p (s q) -> p s q", s=2)[:, :, m:512]
                else:
                    src, dst = psS[:], sexp[:]
                nc.scalar.activation(dst, src, mybir.ActivationFunctionType.Exp)

            def mask_pair(hp, qc, j, sexp):
                m = (j - 4 * qc) * P
                for s in (0, 1):
                    nc.vector.tensor_mul(
                        sexp[:, s * 512 + m : s * 512 + m + P],
                        sexp[:, s * 512 + m : s * 512 + m + P],
                        mask_sb[:],
                    )

            def av_pair(hp, qc, j, nj, sexp, av0, av1):
                m = max(0, (j - 4 * qc) * P)
                for s, av in ((0, av0), (1, av1)):
                    nc.tensor.matmul(
                        av[:, m:512],
                        vt[j][:, 2 * hp + s, :],
                        sexp[:, s * 512 + m : (s + 1) * 512],
                        start=(j == 0),
                        stop=(j == nj - 1),
                        skip_group_check=True,
                    )

            def norm(hp, qc, av0, av1):
                # per-head interleaved chain: each head's broadcast starts as
                # soon as its reciprocal lands, overlapping DVE and GpSimd.
                # (reciprocal_approx_fast misreads PSUM at partition offset
                # 64, so sums stage through SBUF first.)
                qs = slice(qc * 512, (qc + 1) * 512)
                sums = inv_pool.tile([1, 1024], F32, name="sums", tag="sums")
                inv = inv_pool.tile([1, 1024], F32, name="inv", tag="inv")
                invb = invb_pool.tile([64, 1024], F32, name="invb")
                nc.vector.tensor_copy(sums[:, 0:512], av0[64:65, :])
                nc.vector.reciprocal_approx_fast(inv[:, 0:512], sums[:, 0:512])
                nc.vector.tensor_copy(sums[:, 512:1024], av1[64:65, :])
                nc.gpsimd.partition_broadcast(invb[:, 0:512], inv[:, 0:512])
                nc.vector.reciprocal_approx_fast(
                    inv[:, 512:1024], sums[:, 512:1024]
                )
                nc.vector.tensor_mul(yt[hp][0:64, qs], av0[0:64, :], invb[:, 0:512])
                nc.gpsimd.partition_broadcast(
                    invb[:, 512:1024], inv[:, 512:1024]
                )
                nc.vector.tensor_mul(
                    yt[hp][64:128, qs], av1[0:64, :], invb[:, 512:1024]
                )

            # ---- flattened pipeline ----
            LAG = 4
            steps = []
            for qc in range(QC):
                for hp in range(3):
                    nj = 4 * (qc + 1)
                    for j in range(nj):
                        steps.append((hp, qc, j, nj))

            # filler schedule: list of unit thunks per round, consumed one
            # per attention step (extras spill into later steps of the round)
            fillers = {
                0: [
                    lambda: qk_unit(1, 0), lambda: qk_unit(4, 0),
                    lambda: qk_unit(2, 0), lambda: v_unit(0),
                    lambda: v_unit(1), lambda: v_unit(2),
                    lambda: v_unit(3), lambda: qk_unit(5, 0),
                    lambda: qk_unit(0, 1), lambda: qk_unit(3, 1),
                    lambda: v_unit(4), lambda: v_unit(5),
                ],
                1: [
                    lambda: qk_unit(1, 1), lambda: qk_unit(4, 1),
                    lambda: qk_unit(2, 1), lambda: qk_unit(5, 1),
                    lambda: v_unit(6), lambda: v_unit(7),
                    lambda: qk_unit(0, 2), lambda: qk_unit(3, 2),
                    lambda: proj_unit(0, 0), lambda: proj_unit(0, 1),
                    lambda: qk_unit(1, 2), lambda: qk_unit(4, 2),
                    lambda: proj_unit(1, 0), lambda: proj_unit(1, 1),
                    lambda: v_unit(8), lambda: v_unit(9),
                    lambda: proj_unit(2, 0), lambda: proj_unit(2, 1),
                    lambda: qk_unit(2, 2), lambda: qk_unit(5, 2),
                    lambda: proj_unit(3, 0), lambda: proj_unit(3, 1),
                ],
                2: [
                    lambda: v_unit(10), lambda: v_unit(11),
                    lambda: qk_unit(0, 3), lambda: qk_unit(3, 3),
                    lambda: proj_unit(4, 0), lambda: proj_unit(4, 1),
                    lambda: qk_unit(1, 3), lambda: qk_unit(4, 3),
                    lambda: proj_unit(5, 0), lambda: proj_unit(5, 1),
                    lambda: qk_unit(2, 3), lambda: qk_unit(5, 3),
                    lambda: proj_unit(6, 0), lambda: proj_unit(6, 1),
                    lambda: v_unit(12), lambda: v_unit(13),
                    lambda: proj_unit(7, 0), lambda: proj_unit(7, 1),
                    lambda: v_unit(14), lambda: v_unit(15),
                ],
            }
            # spread fillers evenly across the round's steps
            round_first_step = {0: 0, 1: 12, 2: 36, 3: 72}
            round_len = {0: 12, 1: 24, 2: 36, 3: 48}
            # proj fillers for round r-1 must be emitted after the AV-lag
            # queue has popped norm(hp2, r-1): offset the spread by LAG+2.
            step_fillers = {}
            for r, units in fillers.items():
                n_steps = round_len[r]
                for i, u in enumerate(units):
                    s = round_first_step[r] + min(
                        i * n_steps // max(len(units), 1), n_steps - 1
                    )
                    step_fillers.setdefault(s, []).append(u)
            # round 3: place proj(8)/proj(9) right after the hp0/hp1 block
            # ends so the PE stays busy (HAM warm) through those norms;
            # proj(10)/proj(11) move to the epilogue to cover the last norm
            for i, (t_, h_) in enumerate([(8, 0), (8, 1)]):
                step_fillers.setdefault(72 + 16 + LAG + i, []).append(
                    lambda t_=t_, h_=h_: proj_unit(t_, h_)
                )
            for i, (t_, h_) in enumerate([(9, 0), (9, 1)]):
                step_fillers.setdefault(72 + 32 + LAG + i, []).append(
                    lambda t_=t_, h_=h_: proj_unit(t_, h_)
                )

            # prologue: just enough to unlock S(hp0, qc0); V chunks arrive as
            # step-0/1 fillers (AV lags by 3 steps)
            qk_unit(0, 0)
            qk_unit(3, 0)

            # pipeline loop
            pend = []  # (hp, qc, j, nj, sexp, av0, av1)
            block_avs = {}

            def emit_av(rec):
                hp, qc, j, nj, sexp, av0, av1 = rec
                av_pair(hp, qc, j, nj, sexp, av0, av1)
                if j == nj - 1:
                    norm(hp, qc, av0, av1)

            for idx, (hp, qc, j, nj) in enumerate(steps):
                if j == 0:
                    av0 = ps_av.tile([65, 512], F32, tag="av", name=f"av0_{hp}_{qc}")
                    av1 = ps_av.tile([65, 512], F32, tag="av", name=f"av1_{hp}_{qc}")
                    block_avs[(hp, qc)] = (av0, av1)
                av0, av1 = block_avs[(hp, qc)]
                psS = ps_s.tile([P, 1024], F32, tag="s", name="psS")
                s_pair(hp, qc, j, psS)
                sexp = sexp_pool.tile([P, 1024], DT, tag="sexp", name="sexp")
                exp_pair(hp, qc, j, psS, sexp)
                if j - 4 * qc >= 0:
                    mask_pair(hp, qc, j, sexp)
                pend.append((hp, qc, j, nj, sexp, av0, av1))
                if len(pend) > LAG:
                    emit_av(pend.pop(0))
                for u in step_fillers.get(idx, []):
                    u()
            while pend:
                emit_av(pend.pop(0))

            # epilogue: proj(10)/(11) first — they depend only on qc2 norms
            # (long done), so they fill the final norm window and keep the
            # PE clock warm — then the last round's projection
            for t in range(10, 16):
                proj_unit(t, 0)
                proj_unit(t, 1)

    nc.finalize()
    return nc


def shard_inputs(x, w_attn, b_attn, w_proj, b_proj):
    """Host-side prep: slice per core, transpose x, cast to bf16."""
    scale = 1.0 / np.sqrt(D)
    tril = np.tril(np.ones((P, P), np.float32))
    # mask[k_local, q_local] = 1 where k <= q
    mask = tril.T.astype(BF16)
    in_maps = []
    for core in range(8):
        b, half = divmod(core, 2)
        h0 = half * NH
        cq = slice(h0 * D, (h0 + NH) * D)
        ck = slice(C + h0 * D, C + (h0 + NH) * D)
        cv = slice(2 * C + h0 * D, 2 * C + (h0 + NH) * D)
        wq = (w_attn[:, cq] * scale).astype(BF16)
        wk = w_attn[:, ck].astype(BF16)
        # pair-major: [C, 3 pairs, 256] with Q chunk then K chunk per pair
        wqk_pm = np.empty((C, 3, 2 * P), BF16)
        for p_ in range(3):
            wqk_pm[:, p_, 0:P] = wq[:, p_ * P : (p_ + 1) * P]
            wqk_pm[:, p_, P : 2 * P] = wk[:, p_ * P : (p_ + 1) * P]
        bqk = np.concatenate([(b_attn[cq] * scale), b_attn[ck]], axis=0).astype(
            np.float32
        )
        bqk_col = np.ascontiguousarray(bqk.reshape(2 * NH * D // P, P).T)
        wv = w_attn[:, cv].astype(BF16)
        bv = b_attn[cv].astype(BF16)[None, :]
        wp = w_proj[h0 * D : (h0 + NH) * D, :].astype(BF16)
        bp = (b_proj if half == 0 else np.zeros_like(b_proj)).astype(BF16)[None, :]
        xt = np.ascontiguousarray(x[b].T)  # [C, T] fp32
        # [P, QC, CCH, 512] t-block-major
        xt_tb = np.ascontiguousarray(
            xt.reshape(CCH, P, QC, 512).transpose(1, 2, 0, 3)
        )
        in_maps.append(
            {
                "xt": xt_tb.astype(BF16),
                "wqk": np.ascontiguousarray(
                    wqk_pm.reshape(CCH, P, 3, 2 * P).transpose(1, 2, 0, 3)
                ),
                "bqk": bqk_col,
                "wv": np.ascontiguousarray(
                    wv.reshape(CCH, P, NH * D).transpose(1, 0, 2)
                ),
                "bv": bv,
                "wp": np.ascontiguousarray(
                    wp.reshape(NH * D // P, P, C).transpose(1, 0, 2)
                ),
                "bp": bp,
                "mask": mask,
            }
        )
    return in_maps


_NC = None


def _get_nc():
    global _NC
    if _NC is None:
        _NC = build_nc()
    return _NC


def run_sharded(in_maps, trace=False, **kw):
    nc = _get_nc()
    return run_bass_kernel_spmd(nc, in_maps, core_ids=list(range(8)), trace=trace, **kw)


def gather(results):
    out = np.zeros((B, T, C), np.float32)
    for core in range(8):
        b = core // 2
        out[b] += results[core]["out"].astype(np.float32)
    return out


def kernel(x, w_attn, b_attn, w_proj, b_proj):
    x = np.asarray(x, np.float32)
    w_attn = np.asarray(w_attn, np.float32)
    b_attn = np.asarray(b_attn, np.float32)
    w_proj = np.asarray(w_proj, np.float32)
    b_proj = np.asarray(b_proj, np.float32)
    in_maps = shard_inputs(x, w_attn, b_attn, w_proj, b_proj)
    res = run_sharded(in_maps, trace=False)
    return gather(res.results)


# revision 33
# speedup vs baseline: 1.0864x; 1.0088x over previous
"""Causal self-attention (GPT-2 small: B=4, T=2048, C=768, H=12, D=64)
on 8 TRN2 NeuronCores.

Sharding: core i handles batch b = i//2 and head-half = i%2 (6 heads each).
No cross-core collectives; the two half-head partial output projections per
batch are summed on the host during unshard (row-parallel c_proj).

Device kernel (per core, all matmuls bf16, fp32 PSUM accumulation):
  - The two heads of a pair run as CONCURRENT row-tiled S matmuls (K=64 at
    PE row offsets 0/64, tile_position auto-derived) writing the two
    512-col halves of one [128,1024] PSUM tile -> S-gen at full PE rate.
  - One exp per (pair, q-block, k-chunk) covers both heads, causally
    trimmed via a 3-D access pattern.  TensorE (~166 us) and ScalarE exp
    (~115 us) are the two critical engines; DVE/GpSimd/DMA hide.
  - Flattened software-pipelined emission: AV lags S by LAG steps; QKV and
    output-projection matmuls are split into ~1 us filler units woven
    between attention steps (the Tile list scheduler uses program order as
    priority).  Round-3 proj fillers sit right after block boundaries and
    the epilogue so the PE stays busy (HAM stays at full clock) through
    the final softmax normalizations.
  - Softmax normalization: ones-column in V accumulates row sums into PSUM
    partition 64; sums are copied to SBUF (custom-DVE reciprocal misreads
    PSUM at partition offset 64), reciprocal_approx_fast, one
    partition-broadcast per pair, two DVE multiplies into y^T.
  - DMA: all issues on the sync queue (keeping the ACT queue free of
    head-of-line DMA waits); inputs are laid out host-side so every
    transfer is per-partition contiguous at full HBM rate, ordered by
    first use; output is written bf16 (summed in fp32 on the host).
"""

import sys

if "/opt/trn_rl_repo" not in sys.path:
    sys.path.insert(0, "/opt/trn_rl_repo")

import numpy as np
import ml_dtypes

import concourse.bass as bass  # noqa: F401
import concourse.mybir as mybir
from concourse import bacc
from concourse.tile import TileContext
from concourse.bass_utils import run_bass_kernel_spmd

BF16 = ml_dtypes.bfloat16

B, T, C = 4, 2048, 768
H, D = 12, 64
NH = 6  # heads per core
P = 128
TC = T // P  # 16 t-chunks of 128
QC = T // 512  # 4 q-blocks of 512
CCH = C // P  # 6 contraction chunks

DT = mybir.dt.bfloat16
F32 = mybir.dt.float32


def build_nc():
    nc = bacc.Bacc()

    # t-block-major so each 512-col block is one per-partition-contiguous DMA
    xt_d = nc.declare_dram_parameter("xt", [P, QC, CCH, 512], DT, isOutput=False)
    # pair-major: wqk[p, pair, cc, 0:128]=Q chunk, [..., 128:256]=K chunk —
    # per-partition-contiguous per pair so each pair is one full-rate DMA
    wqk_d = nc.declare_dram_parameter("wqk", [P, 3, CCH, 2 * P], DT, isOutput=False)
    bqk_d = nc.declare_dram_parameter("bqk", [P, 2 * NH * D // P], F32, isOutput=False)
    wv_d = nc.declare_dram_parameter("wv", [P, CCH, NH * D], DT, isOutput=False)
    bv_d = nc.declare_dram_parameter("bv", [1, NH * D], DT, isOutput=False)
    wp_d = nc.declare_dram_parameter("wp", [P, NH * D // P, C], DT, isOutput=False)
    bp_d = nc.declare_dram_parameter("bp", [1, C], DT, isOutput=False)
    mask_d = nc.declare_dram_parameter("mask", [P, P], DT, isOutput=False)
    out_d = nc.declare_dram_parameter("out", [T, C], DT, isOutput=True)

    with TileContext(nc) as tc:
        with (
            tc.tile_pool(name="consts", bufs=1) as consts,
            tc.tile_pool(name="sexp", bufs=7) as sexp_pool,
            tc.tile_pool(name="inv", bufs=2) as inv_pool,
            tc.tile_pool(name="invb", bufs=2) as invb_pool,
            tc.tile_pool(name="outp", bufs=4) as outp,
            tc.tile_pool(name="ps_s", bufs=2, space="PSUM") as ps_s,
            tc.tile_pool(name="ps_av", bufs=2, space="PSUM") as ps_av,
            tc.tile_pool(name="ps_mm", bufs=2, space="PSUM") as ps_mm,
        ):
            # ---- input DMAs, one queue (a single InstDMACopy spreads over
            # all 16 SDMA engines; a second ring does not add HBM bandwidth),
            # issued on sync so the ACT queue stays clean for exps.
            # Ordered by first use.
            wqk_sb = consts.tile([P, 3, CCH, 2 * P], DT)
            nc.sync.dma_start(wqk_sb[:, 0], wqk_d[:, 0])
            bqk_sb = consts.tile([P, 2 * NH * D // P], F32)
            nc.sync.dma_start(bqk_sb[:], bqk_d[:])
            xt_sb = consts.tile([P, QC, CCH, 512], DT)
            nc.sync.dma_start(xt_sb[:, 0], xt_d[:, 0])
            nc.sync.dma_start(wqk_sb[:, 1], wqk_d[:, 1])
            nc.sync.dma_start(wqk_sb[:, 2], wqk_d[:, 2])
            mask_sb = consts.tile([P, P], DT)
            nc.sync.dma_start(mask_sb[:], mask_d[:])
            wv_sb = consts.tile([P, CCH, NH * D], DT)
            nc.sync.dma_start(wv_sb[:], wv_d[:])
            bv_sb = consts.tile([1, NH * D], DT)
            nc.sync.dma_start(bv_sb[:], bv_d[:])
            nc.sync.dma_start(xt_sb[:, 1], xt_d[:, 1])
            nc.sync.dma_start(xt_sb[:, 2], xt_d[:, 2])
            nc.sync.dma_start(xt_sb[:, 3], xt_d[:, 3])
            wp_sb = consts.tile([P, NH * D // P, C], DT)
            nc.sync.dma_start(wp_sb[:], wp_d[:])
            bp_sb = consts.tile([1, C], DT)
            nc.sync.dma_start(bp_sb[:], bp_d[:])

            # pre-warm the ACT exp table during the DMA-bound ramp
            warm = consts.tile([1, 8], F32)
            nc.gpsimd.memset(warm[:], 0.0)
            nc.scalar.activation(warm[:], warm[:], mybir.ActivationFunctionType.Exp)

            bvb = consts.tile([P, NH * D], DT)
            nc.gpsimd.partition_broadcast(bvb[:], bv_sb[:])
            bpb = consts.tile([P, C], DT)
            nc.gpsimd.partition_broadcast(bpb[:], bp_sb[:])

            # Q^T/K^T as head-pair tiles [128, T]: head 2p in partitions 0:64,
            # head 2p+1 in partitions 64:128.
            qtp = [consts.tile([P, T], DT, name=f"qtp{p}", tag=f"qtp{p}") for p in range(3)]
            ktp = [consts.tile([P, T], DT, name=f"ktp{p}", tag=f"ktp{p}") for p in range(3)]
            # V per t-chunk, heads side by side with a ones column: [128, 6, 65]
            vt = [consts.tile([P, NH, D + 1], DT, name=f"vt{t}", tag=f"vt{t}") for t in range(TC)]
            for t in range(TC):
                nc.gpsimd.memset(vt[t][:, :, D : D + 1], 1.0)
            # y^T per head-pair [128, T] bf16 (unnormalized until norm step)
            yt = [consts.tile([P, T], DT, name=f"yt{p}", tag=f"yt{p}") for p in range(3)]

            # ---- filler units (QKV projection / output projection) ----
            def qk_unit(fc, r):
                # feature chunk fc: 0..2 -> Q pair fc, 3..5 -> K pair fc-3
                pair, koff = (fc, 0) if fc < 3 else (fc - 3, P)
                pq = ps_mm.tile([P, 512], F32, tag="mm", name="pq")
                for cc in range(CCH):
                    nc.tensor.matmul(
                        pq[:],
                        wqk_sb[:, pair, cc, koff : koff + P],
                        xt_sb[:, r, cc, :],
                        start=(cc == 0),
                        stop=(cc == CCH - 1),
                        skip_group_check=True,
                    )
                dst = qtp[fc] if fc < 3 else ktp[fc - 3]
                nc.vector.tensor_scalar_add(
                    dst[:, r * 512 : (r + 1) * 512],
                    pq[:],
                    bqk_sb[:, fc : fc + 1],
                )

            def v_unit(t):
                pv = ps_mm.tile([P, NH * D], F32, tag="mm", name="pv")
                for cc in range(CCH):
                    nc.tensor.matmul(
                        pv[:],
                        xt_sb[:, t // 4, cc, (t % 4) * P : (t % 4 + 1) * P],
                        wv_sb[:, cc, :],
                        start=(cc == 0),
                        stop=(cc == CCH - 1),
                        skip_group_check=True,
                    )
                nc.vector.tensor_add(
                    vt[t][:, :, 0:D],
                    pv[:].rearrange("p (h d) -> p h d", d=D),
                    bvb[:].rearrange("p (h d) -> p h d", d=D),
                )

            proj_stg = {}

            def proj_unit(t, half):
                # halves share one [128, C] staging tile; half 1 sends the
                # whole row-block as a single contiguous DMA
                c0, c1 = (0, 384) if half == 0 else (384, C)
                pp = ps_mm.tile([P, c1 - c0], F32, tag="mm", name="pp")
                for cp in range(3):
                    nc.tensor.matmul(
                        pp[:],
                        yt[cp][:, t * P : (t + 1) * P],
                        wp_sb[:, cp, c0:c1],
                        start=(cp == 0),
                        stop=(cp == 2),
                        skip_group_check=True,
                    )
                if half == 0:
                    proj_stg[t] = outp.tile([P, C], DT, name="stg", tag="stg")
                stg = proj_stg[t]
                nc.vector.tensor_add(stg[:, c0:c1], pp[:], bpb[:, c0:c1])
                if half == 1:
                    nc.sync.dma_start(out_d[t * P : (t + 1) * P, :], stg[:])

            # ---- attention step pieces ----
            # per-block state: av tiles (h0, h1) and the step list
            def s_pair(hp, qc, j, psS):
                m = max(0, (j - 4 * qc) * P)
                js = slice(j * P, (j + 1) * P)
                qs = slice(qc * 512 + m, (qc + 1) * 512)
                nc.tensor.matmul(
                    psS[:, m:512],
                    ktp[hp][0:64, js],
                    qtp[hp][0:64, qs],
                    start=True,
                    stop=True,
                )
                nc.tensor.matmul(
                    psS[:, 512 + m : 1024],
                    ktp[hp][64:128, js],
                    qtp[hp][64:128, qs],
                    start=True,
                    stop=True,
                )

            def exp_pair(hp, qc, j, psS, sexp):
                m = max(0, (j - 4 * qc) * P)
                if m:
                    src = psS[:].rearrange("p (s q) -> p s q", s=2)[:, :, m:512]
                    dst = sexp[:].rearrange("p (s q) -> p s q", s=2)[:, :, m:512]
                else:
                    src, dst = psS[:], sexp[:]
                nc.scalar.activation(dst, src, mybir.ActivationFunctionType.Exp)

            def mask_pair(hp, qc, j, sexp):
                m = (j - 4 * qc) * P
                for s in (0, 1):
                    nc.vector.tensor_mul(
                        sexp[:, s * 512 + m : s * 512 + m + P],
                        sexp[:, s * 512 + m : s * 512 + m + P],
                        mask_sb[:],
                    )

            def av_pair(hp, qc, j, nj, sexp, av0, av1):
                m = max(0, (j - 4 * qc) * P)
                for s, av in ((0, av0), (1, av1)):
                    nc.tensor.matmul(
                        av[:, m:512],
                        vt[j][:, 2 * hp + s, :],
                        sexp[:, s * 512 + m : (s + 1) * 512],
                        start=(j == 0),
                        stop=(j == nj - 1),
                        skip_group_check=True,
                    )

            def norm(hp, qc, av0, av1):
                # per-head interleaved chain: each head's broadcast starts as
                # soon as its reciprocal lands, overlapping DVE and GpSimd.
                # (reciprocal_approx_fast misreads PSUM at partition offset
                # 64, so sums stage through SBUF first.)
                qs = slice(qc * 512, (qc + 1) * 512)
                sums = inv_pool.tile([1, 1024], F32, name="sums", tag="sums")
                inv = inv_pool.tile([1, 1024], F32, name="inv", tag="inv")
                invb = invb_pool.tile([64, 1024], F32, name="invb")
                nc.vector.tensor_copy(sums[:, 0:512], av0[64:65, :])
                nc.vector.reciprocal_approx_fast(inv[:, 0:512], sums[:, 0:512])
                nc.vector.tensor_copy(sums[:, 512:1024], av1[64:65, :])
                nc.gpsimd.partition_broadcast(invb[:, 0:512], inv[:, 0:512])
                nc.vector.reciprocal_approx_fast(
                    inv[:, 512:1024], sums[:, 512:1024]
                )
                nc.vector.tensor_mul(yt[hp][0:64, qs], av0[0:64, :], invb[:, 0:512])
                nc.gpsimd.partition_broadcast(
                    invb[:, 512:1024], inv[:, 512:1024]
                )
                nc.vector.tensor_mul(
                    yt[hp][64:128, qs], av1[0:64, :], invb[:, 512:1024]
                )

            # ---- flattened pipeline ----
            LAG = 5
            steps = []
            for qc in range(QC):
                for hp in range(3):
                    nj = 4 * (qc + 1)
                    for j in range(nj):
                        steps.append((hp, qc, j, nj))

            # filler schedule: list of unit thunks per round, consumed one
            # per attention step (extras spill into later steps of the round)
            fillers = {
                0: [
                    lambda: qk_unit(1, 0), lambda: qk_unit(4, 0),
                    lambda: qk_unit(2, 0), lambda: v_unit(0),
                    lambda: v_unit(1), lambda: v_unit(2),
                    lambda: v_unit(3), lambda: qk_unit(5, 0),
                    lambda: qk_unit(0, 1), lambda: qk_unit(3, 1),
                    lambda: v_unit(4), lambda: v_unit(5),
                ],
                1: [
                    lambda: qk_unit(1, 1), lambda: qk_unit(4, 1),
                    lambda: qk_unit(2, 1), lambda: qk_unit(5, 1),
                    lambda: v_unit(6), lambda: v_unit(7),
                    lambda: qk_unit(0, 2), lambda: qk_unit(3, 2),
                    lambda: proj_unit(0, 0), lambda: proj_unit(0, 1),
                    lambda: qk_unit(1, 2), lambda: qk_unit(4, 2),
                    lambda: proj_unit(1, 0), lambda: proj_unit(1, 1),
                    lambda: v_unit(8), lambda: v_unit(9),
                    lambda: proj_unit(2, 0), lambda: proj_unit(2, 1),
                    lambda: qk_unit(2, 2), lambda: qk_unit(5, 2),
                    lambda: proj_unit(3, 0), lambda: proj_unit(3, 1),
                ],
                2: [
                    lambda: v_unit(10), lambda: v_unit(11),
                    lambda: qk_unit(0, 3), lambda: qk_unit(3, 3),
                    lambda: proj_unit(4, 0), lambda: proj_unit(4, 1),
                    lambda: qk_unit(1, 3), lambda: qk_unit(4, 3),
                    lambda: proj_unit(5, 0), lambda: proj_unit(5, 1),
                    lambda: qk_unit(2, 3), lambda: qk_unit(5, 3),
                    lambda: proj_unit(6, 0), lambda: proj_unit(6, 1),
                    lambda: v_unit(12), lambda: v_unit(13),
                    lambda: proj_unit(7, 0), lambda: proj_unit(7, 1),
                    lambda: v_unit(14), lambda: v_unit(15),
                ],
            }
            # spread fillers evenly across the round's steps
            round_first_step = {0: 0, 1: 12, 2: 36, 3: 72}
            round_len = {0: 12, 1: 24, 2: 36, 3: 48}
            # proj fillers for round r-1 must be emitted after the AV-lag
            # queue has popped norm(hp2, r-1): offset the spread by LAG+2.
            step_fillers = {}
            for r, units in fillers.items():
                n_steps = round_len[r]
                for i, u in enumerate(units):
                    s = round_first_step[r] + min(
                        i * n_steps // max(len(units), 1), n_steps - 1
                    )
                    step_fillers.setdefault(s, []).append(u)
            # round 3: place proj(8)/proj(9) right after the hp0/hp1 block
            # ends so the PE stays busy (HAM warm) through those norms;
            # proj(10)/proj(11) move to the epilogue to cover the last norm
            for i, (t_, h_) in enumerate([(8, 0), (8, 1)]):
                step_fillers.setdefault(72 + 16 + LAG + i, []).append(
                    lambda t_=t_, h_=h_: proj_unit(t_, h_)
                )
            for i, (t_, h_) in enumerate([(9, 0), (9, 1)]):
                step_fillers.setdefault(72 + 32 + LAG + i, []).append(
                    lambda t_=t_, h_=h_: proj_unit(t_, h_)
                )

            # prologue: just enough to unlock S(hp0, qc0); V chunks arrive as
            # step-0/1 fillers (AV lags by 3 steps)
            qk_unit(0, 0)
            qk_unit(3, 0)

            # pipeline loop
            pend = []  # (hp, qc, j, nj, sexp, av0, av1)
            block_avs = {}

            def emit_av(rec):
                hp, qc, j, nj, sexp, av0, av1 = rec
                av_pair(hp, qc, j, nj, sexp, av0, av1)
                if j == nj - 1:
                    norm(hp, qc, av0, av1)

            for idx, (hp, qc, j, nj) in enumerate(steps):
                if j == 0:
                    av0 = ps_av.tile([65, 512], F32, tag="av", name=f"av0_{hp}_{qc}")
                    av1 = ps_av.tile([65, 512], F32, tag="av", name=f"av1_{hp}_{qc}")
                    block_avs[(hp, qc)] = (av0, av1)
                av0, av1 = block_avs[(hp, qc)]
                psS = ps_s.tile([P, 1024], F32, tag="s", name="psS")
                s_pair(hp, qc, j, psS)
                sexp = sexp_pool.tile([P, 1024], DT, tag="sexp", name="sexp")
                exp_pair(hp, qc, j, psS, sexp)
                if j - 4 * qc >= 0:
                    mask_pair(hp, qc, j, sexp)
                pend.append((hp, qc, j, nj, sexp, av0, av1))
                if len(pend) > LAG:
                    emit_av(pend.pop(0))
                for u in step_fillers.get(idx, []):
                    u()
            while pend:
                emit_av(pend.pop(0))

            # epilogue: proj(10)/(11) first — they depend only on qc2 norms
            # (long done), so they fill the final norm window and keep the
            # PE clock warm — then the last round's projection
            for t in range(10, 16):
                proj_unit(t, 0)
                proj_unit(t, 1)

    nc.finalize()
    return nc


def shard_inputs(x, w_attn, b_attn, w_proj, b_proj):
    """Host-side prep: slice per core, transpose x, cast to bf16."""
    scale = 1.0 / np.sqrt(D)
    tril = np.tril(np.ones((P, P), np.float32))
    # mask[k_local, q_local] = 1 where k <= q
    mask = tril.T.astype(BF16)
    in_maps = []
    for core in range(8):
        b, half = divmod(core, 2)
        h0 = half * NH
        cq = slice(h0 * D, (h0 + NH) * D)
        ck = slice(C + h0 * D, C + (h0 + NH) * D)
        cv = slice(2 * C + h0 * D, 2 * C + (h0 + NH) * D)
        wq = (w_attn[:, cq] * scale).astype(BF16)
        wk = w_attn[:, ck].astype(BF16)
        # pair-major: [C, 3 pairs, 256] with Q chunk then K chunk per pair
        wqk_pm = np.empty((C, 3, 2 * P), BF16)
        for p_ in range(3):
            wqk_pm[:, p_, 0:P] = wq[:, p_ * P : (p_ + 1) * P]
            wqk_pm[:, p_, P : 2 * P] = wk[:, p_ * P : (p_ + 1) * P]
        bqk = np.concatenate([(b_attn[cq] * scale), b_attn[ck]], axis=0).astype(
            np.float32
        )
        bqk_col = np.ascontiguousarray(bqk.reshape(2 * NH * D // P, P).T)
        wv = w_attn[:, cv].astype(BF16)
        bv = b_attn[cv].astype(BF16)[None, :]
        wp = w_proj[h0 * D : (h0 + NH) * D, :].astype(BF16)
        bp = (b_proj if half == 0 else np.zeros_like(b_proj)).astype(BF16)[None, :]
        xt = np.ascontiguousarray(x[b].T)  # [C, T] fp32
        # [P, QC, CCH, 512] t-block-major
        xt_tb = np.ascontiguousarray(
            xt.reshape(CCH, P, QC, 512).transpose(1, 2, 0, 3)
        )
        in_maps.append(
            {
                "xt": xt_tb.astype(BF16),
                "wqk": np.ascontiguousarray(
                    wqk_pm.reshape(CCH, P, 3, 2 * P).transpose(1, 2, 0, 3)
                ),
                "bqk": bqk_col,
                "wv": np.ascontiguousarray(
                    wv.reshape(CCH, P, NH * D).transpose(1, 0, 2)
                ),
                "bv": bv,
                "wp": np.ascontiguousarray(
                    wp.reshape(NH * D // P, P, C).transpose(1, 0, 2)
                ),
                "bp": bp,
                "mask": mask,
            }
        )
    return in_maps


_NC = None


def _get_nc():
    global _NC
    if _NC is None:
        _NC = build_nc()
    return _NC


def run_sharded(in_maps, trace=False, **kw):
    nc = _get_nc()
    return run_bass_kernel_spmd(nc, in_maps, core_ids=list(range(8)), trace=trace, **kw)


def gather(results):
    out = np.zeros((B, T, C), np.float32)
    for core in range(8):
        b = core // 2
        out[b] += results[core]["out"].astype(np.float32)
    return out


def kernel(x, w_attn, b_attn, w_proj, b_proj):
    x = np.asarray(x, np.float32)
    w_attn = np.asarray(w_attn, np.float32)
    b_attn = np.asarray(b_attn, np.float32)
    w_proj = np.asarray(w_proj, np.float32)
    b_proj = np.asarray(b_proj, np.float32)
    in_maps = shard_inputs(x, w_attn, b_attn, w_proj, b_proj)
    res = run_sharded(in_maps, trace=False)
    return gather(res.results)
